# revision 1
# baseline (speedup 1.0000x reference)
"""Trainium2 Bass kernel for 2-layer bidirectional LSTM (B=1024,S=256,F=16,H=64).

Sharding: batch data-parallel across 8 cores (128 batch rows each), weights
replicated. Per core, gate-major layout: gates on partitions, batch on free.

Per direction the 4H=256 gate preactivations are computed as two PSUM tiles
  X = [f;i] (sigmoid), Y = [o;g] (tanh)
via accumulating matmuls (input projection + recurrent projection). The
h-state is stored scaled: h_stored = 2*h_true = (1+tanh(o))*tanh(c), with the
0.5 compensation folded into every consumer weight matrix on the host. This
lets one Sigmoid ACT op and one Tanh ACT op cover all four gates, with the
per-gate bias applied through the ACT bias operand (per-partition vector).

dir f state lives on partitions 0:64, dir r on 64:128, so the layer-0 output
history buffer h0_buf[128, S*B] is directly the layer-1 input, and the two
directions' matmuls occupy disjoint PE row groups (concurrent).
"""
import os
import numpy as np

H = 64
B = 128          # batch per core
S = 256
F = 16
NCORES = 8
FULL_B = 1024
C_OUT = 3

_f32 = None  # set lazily (mybir import)


def _prep_weights(w_ih, w_hh, b_ih, b_hh, scale_in, scale_h):
    """lhsT stacks for X=[f;i], Y=[o;g]; returns dict of host arrays."""
    w_ih = np.asarray(w_ih, np.float32)
    w_hh = np.asarray(w_hh, np.float32)
    b = (np.asarray(b_ih, np.float32) + np.asarray(b_hh, np.float32))
    permX = np.r_[np.arange(64, 128), np.arange(0, 64)]       # [f; i]
    permY = np.r_[np.arange(192, 256), np.arange(128, 192)]   # [o; g]
    out = {}
    # Y stack: o-gate rows pre-scaled by 0.5 so the Tanh ACT yields tanh(o/2),
    # hence 1+tanh(o/2) = 2*sigmoid(o).
    rsX = np.ones((128, 1), np.float32)
    rsY = np.ones((128, 1), np.float32); rsY[0:64] = 0.5
    for name, perm, rs in (("X", permX, rsX), ("Y", permY, rsY)):
        out[f"ih_{name}"] = np.ascontiguousarray((scale_in * rs * w_ih[perm]).T)  # [din,128]
        out[f"hh_{name}"] = np.ascontiguousarray((scale_h * rs * w_hh[perm]).T)   # [64,128]
        out[f"b_{name}"] = np.ascontiguousarray(rs[:, 0] * b[perm])                # [128]
    return out


def _host_prep(inputs):
    """Build all DRAM-side arrays shared by every core (weights) and the
    per-core xT slabs."""
    d = {}
    l0f = _prep_weights(inputs["w_ih_l0"], inputs["w_hh_l0"],
                        inputs["b_ih_l0"], inputs["b_hh_l0"], 1.0, 0.5)
    l0r = _prep_weights(inputs["w_ih_l0r"], inputs["w_hh_l0r"],
                        inputs["b_ih_l0r"], inputs["b_hh_l0r"], 1.0, 0.5)
    l1f = _prep_weights(inputs["w_ih_l1"], inputs["w_hh_l1"],
                        inputs["b_ih_l1"], inputs["b_hh_l1"], 0.5, 0.5)
    l1r = _prep_weights(inputs["w_ih_l1r"], inputs["w_hh_l1r"],
                        inputs["b_ih_l1r"], inputs["b_hh_l1r"], 0.5, 0.5)

    for nm in ("X", "Y"):
        hh0 = np.zeros((128, 128), np.float32)
        hh0[0:64] = l0f[f"hh_{nm}"]
        hh0[64:128] = l0r[f"hh_{nm}"]
        d[f"hh0{nm}"] = hh0
        hh1 = np.zeros((128, 128), np.float32)
        hh1[0:64] = l1f[f"hh_{nm}"]
        hh1[64:128] = l1r[f"hh_{nm}"]
        d[f"hh1{nm}"] = hh1
        ih0 = np.zeros((128, 128), np.float32)
        ih0[0:F] = l0f[f"ih_{nm}"]
        ih0[64:64 + F] = l0r[f"ih_{nm}"]
        d[f"ih0{nm}"] = ih0
        d[f"ih1{nm}f"] = l1f[f"ih_{nm}"]
        d[f"ih1{nm}r"] = l1r[f"ih_{nm}"]

    biases = np.zeros((128, 8), np.float32)
    for li, (lf, lr) in enumerate(((l0f, l0r), (l1f, l1r))):
        for di, wp in enumerate((lf, lr)):
            for si, nm in enumerate(("X", "Y")):
                biases[:, li * 4 + di * 2 + si] = wp[f"b_{nm}"]
    d["biases"] = biases
    d["fcT"] = np.ascontiguousarray(
        (0.5 * np.asarray(inputs["fc_w"], np.float32)).T)           # [128, 3]
    d["fcb"] = np.asarray(inputs["fc_b"], np.float32).reshape(C_OUT, 1)
    return d


def _host_xT(x_core):
    """x_core [B, S, F] -> xT [F, S*B], col = t*B + b."""
    return np.ascontiguousarray(
        np.asarray(x_core, np.float32).transpose(2, 1, 0).reshape(F, -1))


def _patch_tile_drain():
    """This container's walrus rejects instructions carrying multiple sync
    waits ("Too many sync wait commands") — chunk the kernel-tail drain's
    global-clock waits into one drain instruction per semaphore."""
    import concourse.tile as tile
    from concourse.vector_clock import ScopedClock, VectorClock
    if getattr(tile.TileContext, "_drain_patched", False):
        return
    def patched_drain(self, tick_clock, wait_clock):
        gc = tick_clock.global_clock
        n = len(gc)
        procs = [i for i in range(n) if gc[i] > 0]
        chunks = [[p] for p in procs] or [[]]
        for ch in chunks:
            vec = [0] * n
            for p in ch:
                vec[p] = gc[p]
            d = self.nc.sync.drain()
            wait_clock.add_sem_waits(d.ins, ScopedClock({None: VectorClock(vec)}))
        self.nc.all_engine_barrier()
        popped = self.nc._tile_sem_poison_stack.pop()
        assert popped is self._sem_poison
        self.nc.clear_and_free_semaphores(list(self.sems.allocated().values()))
        self.nc.all_engine_barrier()
    tile.TileContext._drain_and_barrier = patched_drain
    tile.TileContext._drain_patched = True


def _split_multi_waits(nc, mybir):
    """This walrus build rejects instructions with more than one sync wait.
    Hoist extra waits onto same-engine NoOp instructions inserted immediately
    before the owning instruction (identical semantics: the engine is
    sequential, so waiting on the prior instruction slot is equivalent)."""
    for f in nc.m.functions:
        for bb in f.blocks:
            out = []
            changed = False
            for inst in bb.instructions:
                si = inst.sync_info
                waits = list(si.on_wait) if si is not None else []
                if len(waits) > 1:
                    changed = True
                    for w in waits[:-1]:
                        nop = mybir.InstNoOp(
                            name=nc.get_next_instruction_name(), ins=[], outs=[])
                        nop.engine = inst.engine
                        nop.sync_info = mybir.SyncInfo(on_wait=[w], on_update=[])
                        out.append(nop)
                    inst.sync_info = mybir.SyncInfo(
                        on_wait=[waits[-1]], on_update=list(si.on_update))
                out.append(inst)
            if changed:
                bb.instructions = out


def build_nc(s_steps=S, use_f32r=False):
    import concourse.bass as bass
    import concourse.tile as tile
    from concourse import mybir
    _patch_tile_drain()

    f32 = mybir.dt.float32
    f32r = mybir.dt.float32r
    AF = mybir.ActivationFunctionType
    ALU = mybir.AluOpType

    def mmcast(ap):
        return ap.bitcast(f32r) if use_f32r else ap

    nc = bass.Bass("TRN2", target_bir_lowering=False, debug=False)

    xT_d = nc.dram_tensor("xT", [F, s_steps * B], f32, kind="ExternalInput")
    wnames = ["hh0X", "hh0Y", "hh1X", "hh1Y", "ih0X", "ih0Y",
              "ih1Xf", "ih1Xr", "ih1Yf", "ih1Yr"]
    wd = {n: nc.dram_tensor(n, [128, 128], f32, kind="ExternalInput")
          for n in wnames}
    bias_d = nc.dram_tensor("biases", [128, 8], f32, kind="ExternalInput")
    fcT_d = nc.dram_tensor("fcT", [128, C_OUT], f32, kind="ExternalInput")
    fcb_d = nc.dram_tensor("fcb", [C_OUT, 1], f32, kind="ExternalInput")
    out_d = nc.dram_tensor("out", [C_OUT, B], f32, kind="ExternalOutput")

    with tile.TileContext(nc) as tc:
        with tc.tile_pool(name="pers", bufs=1) as pers, \
             tc.tile_pool(name="xin", bufs=6) as xin, \
             tc.tile_pool(name="gat", bufs=3) as gat, \
             tc.tile_pool(name="tmp", bufs=3) as tmp, \
             tc.tile_pool(name="ps", bufs=4, space="PSUM") as ps:

            # --- persistent state ---
            h0_buf = pers.tile([128, s_steps * B], f32, tag="h0buf")
            h1_state = pers.tile([128, B], f32, tag="h1s")
            h1_last = pers.tile([128, B], f32, tag="h1l")
            cst = {"f": pers.tile([64, B], f32, tag="cf", name="cf"),
                   "r": pers.tile([64, B], f32, tag="cr", name="cr")}

            # --- weights to SBUF ---
            wsb = {}
            for n in wnames:
                t = pers.tile([128, 128], f32, tag=f"w_{n}", name=f"w_{n}")
                nc.sync.dma_start(out=t[:], in_=wd[n][:])
                wsb[n] = t
            bias_sb = pers.tile([128, 8], f32, tag="bias")
            nc.sync.dma_start(out=bias_sb[:], in_=bias_d[:])
            fcT_sb = pers.tile([128, C_OUT], f32, tag="fcT")
            nc.sync.dma_start(out=fcT_sb[:], in_=fcT_d[:])
            fcb_sb = pers.tile([C_OUT, 1], f32, tag="fcb")
            nc.sync.dma_start(out=fcb_sb[:], in_=fcb_d[:])

            def ts(t):
                return slice(t * B, (t + 1) * B)

            def step(layer, s, dir_, prev_written):
                """One scan step for one direction."""
                di = 0 if dir_ == "f" else 1
                t_proc = s if dir_ == "f" else (s_steps - 1 - s)
                lo, hi = (0, 64) if dir_ == "f" else (64, 128)

                pX = ps.tile([128, B], f32, tag="pX")
                pY = ps.tile([128, B], f32, tag="pY")

                # ---- input projection ----
                if layer == 0:
                    xt = xin.tile([128, B], f32, tag=f"x{dir_}")
                    nc.sync.dma_start(out=xt[lo:lo + F, :],
                                      in_=xT_d[:, ts(t_proc)])
                    rhs_in = xt[lo:lo + F, :]
                    lX, lY = wsb["ih0X"][lo:lo + F, :], wsb["ih0Y"][lo:lo + F, :]
                else:
                    rhs_in = h0_buf[:, ts(t_proc)]
                    sfx = dir_
                    lX, lY = wsb[f"ih1X{sfx}"][:], wsb[f"ih1Y{sfx}"][:]
                first = True
                nc.tensor.matmul(pX[:], mmcast(lX), mmcast(rhs_in),
                                 start=first, stop=(prev_written is None))
                nc.tensor.matmul(pY[:], mmcast(lY), mmcast(rhs_in),
                                 start=first, stop=(prev_written is None))

                # ---- recurrent projection ----
                if prev_written is not None:
                    h_prev = prev_written  # AP [64, B] at partitions lo:hi
                    whX = wsb[f"hh{layer}X"][lo:hi, :]
                    whY = wsb[f"hh{layer}Y"][lo:hi, :]
                    nc.tensor.matmul(pX[:], mmcast(whX), mmcast(h_prev),
                                     start=False, stop=True)
                    nc.tensor.matmul(pY[:], mmcast(whY), mmcast(h_prev),
                                     start=False, stop=True)

                bX = bias_sb[:, layer * 4 + di * 2: layer * 4 + di * 2 + 1]
                bY = bias_sb[:, layer * 4 + di * 2 + 1: layer * 4 + di * 2 + 2]
                sfi = gat.tile([128, B], f32, tag=f"sfi{dir_}")
                tog = gat.tile([128, B], f32, tag=f"tog{dir_}")
                nc.scalar.activation(sfi[:], pX[:], AF.Sigmoid, bias=bX)
                nc.scalar.activation(tog[:], pY[:], AF.Tanh, bias=bY)

                Cd = cst[dir_]
                t1 = tmp.tile([64, B], f32, tag=f"t1{dir_}")
                t2 = tmp.tile([64, B], f32, tag=f"t2{dir_}")
                if prev_written is not None:
                    nc.vector.tensor_mul(t1[:], sfi[0:64, :], Cd[:])
                    nc.vector.tensor_mul(t2[:], sfi[64:128, :], tog[64:128, :])
                    nc.vector.tensor_add(Cd[:], t1[:], t2[:])
                else:
                    nc.vector.tensor_mul(Cd[:], sfi[64:128, :], tog[64:128, :])
                tcv = tmp.tile([64, B], f32, tag=f"tc{dir_}")
                qv = tmp.tile([64, B], f32, tag=f"q{dir_}")
                nc.scalar.activation(tcv[:], Cd[:], AF.Tanh)
                nc.scalar.add(qv[:], tog[0:64, :], 1.0)

                # ---- h write (h_stored = 2h) ----
                if layer == 0:
                    dst = h0_buf[lo:hi, ts(t_proc)]
                    nc.vector.tensor_mul(dst, qv[:], tcv[:])
                    return dst
                else:
                    if dir_ == "f" and s == s_steps - 1:
                        dst = h1_last[0:64, :]
                        nc.vector.tensor_mul(dst, qv[:], tcv[:])
                        return dst
                    dst = h1_state[lo:hi, :]
                    nc.vector.tensor_mul(dst, qv[:], tcv[:])
                    if dir_ == "r" and s == 0:
                        nc.vector.tensor_mul(h1_last[64:128, :], qv[:], tcv[:])
                    return dst

            for layer in (0, 1):
                prev = {"f": None, "r": None}
                for s in range(s_steps):
                    for dir_ in ("f", "r"):
                        prev[dir_] = step(layer, s, dir_, prev[dir_])

            # ---- fc head ----
            pfc = ps.tile([128, B], f32, tag="pX")
            nc.tensor.matmul(pfc[0:C_OUT, :], mmcast(fcT_sb[:]),
                             mmcast(h1_last[:]), start=True, stop=True)
            osb = gat.tile([C_OUT, B], f32, tag="osb")
            nc.scalar.activation(osb[:], pfc[0:C_OUT, :], AF.Identity,
                                 bias=fcb_sb[:, 0:1])
            nc.sync.dma_start(out=out_d[:], in_=osb[:])

    _split_multi_waits(nc, mybir)
    return nc


_cached = {}


def kernel(**inputs):
    from concourse.bass_utils import run_bass_kernel_spmd

    key = "nc"
    if key not in _cached:
        _cached[key] = build_nc(S, use_f32r=False)
    nc = _cached[key]

    shared = _host_prep(inputs)
    x = np.asarray(inputs["x"], np.float32)
    in_maps = []
    for c in range(NCORES):
        m = dict(shared)
        m["xT"] = _host_xT(x[c * B:(c + 1) * B])
        in_maps.append(m)

    res = run_bass_kernel_spmd(nc, in_maps, list(range(NCORES)))
    out = np.concatenate([r["out"].T for r in res.results], axis=0)
    return np.ascontiguousarray(out.astype(np.float32))



# revision 7
# speedup vs baseline: 302.2838x; 302.2838x over previous
"""Trainium2 Bass kernel for 2-layer bidirectional LSTM (B=1024,S=256,F=16,H=64).

Sharding: batch data-parallel across 8 cores (128 batch rows each), weights
replicated. Per core the 128-row batch is split into two 64-row halves that
run as independent, staggered recurrence chains to hide per-step dependency
latency behind engine work.

All four gates of both directions are evaluated with a SINGLE Tanh ACT op per
half-step using sigma(a) = (1 + tanh(a/2)) / 2: the i/f/o gate rows of every
weight matrix and bias are pre-scaled by 1/2 on the host, so the PSUM gate
tile [128, 256] = [X_f | X_r | Y_f | Y_r] (X = [f;i] rows, Y = [o;g] rows)
goes through one tanh. With c stored as 2c and h stored as 2h:

    cb = (t_i + 1) * t_g            (gpsimd: mul + add)
    ca = (t_f + 1) * c2             (DVE scalar_tensor_tensor)
    c2' = 0.5 * ca + cb             (DVE scalar_tensor_tensor)
    tc = tanh(0.5 * c2')            (ACT, scale operand)
    h2 = (t_o + 1) * tc             (DVE scalar_tensor_tensor, fp16 out)

The 0.5 compensation for the doubled h is folded into every consumer weight
matrix on the host. Matmul inputs (x, h, weights) are fp16: 1 PE cycle/row
vs fp32's 4. Biases ride in the matmuls via a constant-one row appended to
the layer-0 x slab (K=17) and to the layer-1 h state tile (K=65), so the ACT
needs no bias operand and gates of both directions share one instruction.
"""
import numpy as np

H = 64
B = 128          # batch per core
HB = 64          # batch half per chain
S = 256
F = 16
CT = 32          # timesteps per streamed x chunk
NCORES = 8
FULL_B = 1024
C_OUT = 3


def _prep_dir(w_ih, w_hh, b_ih, b_hh, layer):
    """Per-direction lhsT stacks (fp16) with the all-tanh row scaling.

    PyTorch gate row order i,f,g,o. X-stack = [f;i], Y-stack = [o;g].
    Row scale 1/2 on i,f,o (sigma->tanh argument halving); g unscaled.
    Input-side scale 1/2 on everything that consumes a doubled h: all w_hh,
    and layer-1's w_ih (its input is the doubled h0).
    """
    w_ih = np.asarray(w_ih, np.float64)
    w_hh = np.asarray(w_hh, np.float64)
    b = np.asarray(b_ih, np.float64) + np.asarray(b_hh, np.float64)
    permX = np.r_[64:128, 0:64]        # [f; i]
    permY = np.r_[192:256, 128:192]    # [o; g]
    rsX = np.full((128, 1), 0.5)
    rsY = np.concatenate([np.full((64, 1), 0.5), np.ones((64, 1))])
    ih_scale = 1.0 if layer == 0 else 0.5
    out = {}
    for nm, perm, rs in (("X", permX, rsX), ("Y", permY, rsY)):
        wi = (ih_scale * rs * w_ih[perm]).T        # [din, 128]
        wh = (0.5 * rs * w_hh[perm]).T             # [64, 128]
        bb = (rs[:, 0] * b[perm])[None, :]         # [1, 128]
        if layer == 0:
            out[f"w{nm}"] = np.ascontiguousarray(
                np.concatenate([wi, bb], axis=0)).astype(np.float16)   # [17,128]
            out[f"u{nm}"] = np.ascontiguousarray(wh).astype(np.float16)  # [64,128]
        else:
            out[f"w{nm}"] = np.ascontiguousarray(wi).astype(np.float16)  # [128,128]
            out[f"u{nm}"] = np.ascontiguousarray(
                np.concatenate([wh, bb], axis=0)).astype(np.float16)   # [65,128]
    return out


def _host_prep(inputs):
    """Shared (weight) DRAM arrays, replicated to every core."""
    d = {}
    for layer in (0, 1):
        for di, suf in enumerate(("", "r")):
            p = _prep_dir(inputs[f"w_ih_l{layer}{suf}"],
                          inputs[f"w_hh_l{layer}{suf}"],
                          inputs[f"b_ih_l{layer}{suf}"],
                          inputs[f"b_hh_l{layer}{suf}"], layer)
            dch = "f" if di == 0 else "r"
            for nm in ("X", "Y"):
                d[f"w{layer}{nm}{dch}"] = p[f"w{nm}"]
                d[f"u{layer}{nm}{dch}"] = p[f"u{nm}"]
    # Layer-0 recurrent weights dir-stacked on partitions so the dir-r
    # matmul's lhsT shares the rhs base partition (h0_buf rows 64:128).
    for nm in ("X", "Y"):
        d[f"u0{nm}c"] = np.ascontiguousarray(
            np.concatenate([d.pop(f"u0{nm}f"), d.pop(f"u0{nm}r")], axis=0))
    d["fcT"] = np.ascontiguousarray(
        (0.5 * np.asarray(inputs["fc_w"], np.float64)).T).astype(np.float16)
    d["fcb"] = np.asarray(inputs["fc_b"], np.float32).reshape(C_OUT, 1)
    return d


def _host_xT(x_core):
    """x_core [B, S, F] -> [F+1, S*B] fp16, col = t*B + b, last row = 1."""
    xt = np.asarray(x_core, np.float32).transpose(2, 1, 0).reshape(F, -1)
    ones = np.ones((1, xt.shape[1]), np.float32)
    return np.concatenate([xt, ones], axis=0).astype(np.float16)


def _patch_tile_drain():
    """This container's walrus rejects instructions carrying multiple sync
    waits ("Too many sync wait commands") — chunk the kernel-tail drain's
    global-clock waits into one drain instruction per semaphore."""
    import concourse.tile as tile
    from concourse.vector_clock import ScopedClock, VectorClock
    if getattr(tile.TileContext, "_drain_patched", False):
        return
    def patched_drain(self, tick_clock, wait_clock):
        gc = tick_clock.global_clock
        n = len(gc)
        procs = [i for i in range(n) if gc[i] > 0]
        chunks = [[p] for p in procs] or [[]]
        for ch in chunks:
            vec = [0] * n
            for p in ch:
                vec[p] = gc[p]
            d = self.nc.sync.drain()
            wait_clock.add_sem_waits(d.ins, ScopedClock({None: VectorClock(vec)}))
        self.nc.all_engine_barrier()
        popped = self.nc._tile_sem_poison_stack.pop()
        assert popped is self._sem_poison
        self.nc.clear_and_free_semaphores(list(self.sems.allocated().values()))
        self.nc.all_engine_barrier()
    tile.TileContext._drain_and_barrier = patched_drain
    tile.TileContext._drain_patched = True


def _split_multi_waits(nc, mybir):
    """This walrus build rejects instructions with more than one sync wait.
    Hoist extra waits onto same-engine NoOp instructions inserted immediately
    before the owning instruction (identical semantics: the engine is
    sequential, so waiting on the prior instruction slot is equivalent)."""
    for f in nc.m.functions:
        for bb in f.blocks:
            out = []
            changed = False
            for inst in bb.instructions:
                si = inst.sync_info
                waits = list(si.on_wait) if si is not None else []
                if len(waits) > 1:
                    changed = True
                    for w in waits[:-1]:
                        nop = mybir.InstNoOp(
                            name=nc.get_next_instruction_name(), ins=[], outs=[])
                        nop.engine = inst.engine
                        nop.sync_info = mybir.SyncInfo(on_wait=[w], on_update=[])
                        out.append(nop)
                    inst.sync_info = mybir.SyncInfo(
                        on_wait=[waits[-1]], on_update=list(si.on_update))
                out.append(inst)
            if changed:
                bb.instructions = out


def build_nc(s_steps=S):
    import concourse.bass as bass
    import concourse.tile as tile
    from concourse import mybir
    _patch_tile_drain()

    f32 = mybir.dt.float32
    f16 = mybir.dt.float16
    AF = mybir.ActivationFunctionType
    ALU = mybir.AluOpType

    ct = min(CT, s_steps)
    n_ch = s_steps // ct
    nc = bass.Bass("TRN2", target_bir_lowering=False, debug=False)

    xT_d = nc.dram_tensor("xT", [F + 1, s_steps * B], f16, kind="ExternalInput")
    wnames = ([f"w0{nm}{dc}" for nm in "XY" for dc in "fr"]
              + [f"u0{nm}c" for nm in "XY"]
              + [f"w1{nm}{dc}" for nm in "XY" for dc in "fr"]
              + [f"u1{nm}{dc}" for nm in "XY" for dc in "fr"])
    wshape = {"w0": [F + 1, 128], "u0": [128, 128], "w1": [128, 128], "u1": [65, 128]}
    wd = {n: nc.dram_tensor(n, wshape[n[:2]], f16, kind="ExternalInput")
          for n in wnames}
    fcT_d = nc.dram_tensor("fcT", [128, C_OUT], f16, kind="ExternalInput")
    fcb_d = nc.dram_tensor("fcb", [C_OUT, 1], f32, kind="ExternalInput")
    out_d = nc.dram_tensor("out", [C_OUT, B], f32, kind="ExternalOutput")

    with tile.TileContext(nc) as tc:
        with tc.tile_pool(name="pers", bufs=1) as pers, \
             tc.tile_pool(name="xch", bufs=3) as xch, \
             tc.tile_pool(name="wk", bufs=2) as wk, \
             tc.tile_pool(name="ps", bufs=2, space="PSUM") as ps:

            h0_buf = pers.tile([128, s_steps * B], f16, tag="h0buf", name="h0_buf")
            h1 = [pers.tile([65, B], f16, tag=f"h1_{hf}", name=f"h1{hf}")
                  for hf in (0, 1)]
            cst = [pers.tile([64, B], f32, tag=f"c_{hf}", name=f"c{hf}")
                   for hf in (0, 1)]
            h1_last = pers.tile([128, B], f16, tag="h1l", name="h1_last")

            wsb = {}
            for n in wnames:
                t = pers.tile(wshape[n[:2]], f16, tag=f"w_{n}", name=f"w_{n}")
                nc.sync.dma_start(out=t[:], in_=wd[n][:])
                wsb[n] = t
            fcT_sb = pers.tile([128, C_OUT], f16, tag="fcT", name="fcT_sb")
            nc.sync.dma_start(out=fcT_sb[:], in_=fcT_d[:])
            fcb_sb = pers.tile([C_OUT, 1], f32, tag="fcb", name="fcb_sb")
            nc.sync.dma_start(out=fcb_sb[:], in_=fcb_d[:])

            # --- x chunk streaming (layer 0 only), one stream per direction.
            chunks = {}

            def load_chunk(dc, k):
                t = xch.tile([F + 1, ct * B], f16, tag=f"x{dc}", name=f"x{dc}{k}")
                if dc == "f":
                    lo = k * ct * B
                else:
                    lo = (s_steps - (k + 1) * ct) * B
                nc.sync.dma_start(out=t[:], in_=xT_d[:, lo:lo + ct * B])
                chunks[dc, k] = t

            def x_rhs(dc, t_proc, hf):
                k = t_proc // ct if dc == "f" else (s_steps - 1 - t_proc) // ct
                ch = chunks[dc, k]
                base = k * ct if dc == "f" else s_steps - (k + 1) * ct
                off = (t_proc - base) * B + hf * HB
                return ch[:, off:off + HB]

            load_chunk("f", 0)
            load_chunk("r", 0)
            if n_ch > 1:
                load_chunk("f", 1)
                load_chunk("r", 1)

            # Column layout inside the gate PSUM tile [128, 4*HB]:
            # [X_f | X_r | Y_f | Y_r], X rows = [f;i], Y rows = [o;g].
            def phase_mm(layer, s, hf):
                # One PSUM accumulation group open at a time: each region's
                # ih (start) matmul is immediately followed by its hh (stop)
                # matmul — interleaved open groups in one bank lose the
                # earlier contribution.
                G = ps.tile([128, 4 * HB], f32, tag=f"G{hf}", name=f"G{hf}_{layer}_{s}")
                first_step = s == 0
                for di, dc in enumerate(("f", "r")):
                    t_proc = s if dc == "f" else s_steps - 1 - s
                    t_prev = s - 1 if dc == "f" else s_steps - s
                    for ni, nm in enumerate(("X", "Y")):
                        dst = G[:, (2 * ni + di) * HB:(2 * ni + di + 1) * HB]
                        if layer == 0:
                            rhs = x_rhs(dc, t_proc, hf)
                        else:
                            rhs = h0_buf[:, t_proc * B + hf * HB:
                                         t_proc * B + hf * HB + HB]
                        only = layer == 0 and first_step
                        nc.tensor.matmul(dst, wsb[f"w{layer}{nm}{dc}"][:], rhs,
                                         start=True, stop=only)
                        if only:
                            continue
                        if layer == 0:
                            rhs2 = h0_buf[di * 64:(di + 1) * 64,
                                          t_prev * B + hf * HB:
                                          t_prev * B + hf * HB + HB]
                            lhsT = wsb[f"u0{nm}c"][di * 64:(di + 1) * 64, :]
                        else:
                            rhs2 = h1[hf][:, di * HB:(di + 1) * HB]
                            lhsT = wsb[f"u1{nm}{dc}"][:]
                        nc.tensor.matmul(dst, lhsT, rhs2,
                                         start=False, stop=True)
                return G

            def phase_gates(G, hf):
                T = wk.tile([128, 4 * HB], f32, tag=f"T{hf}", name=f"T{hf}")
                nc.scalar.activation(T[:], G[:], AF.Tanh)
                return T

            def phase_cup(T, hf):
                # gpsimd: u = t_g + t_i * t_g  (the (t_i+1)*t_g term)
                ti = T[64:128, 0:B]
                tg = T[64:128, B:2 * B]
                # m lives on partitions 64:128: tensor-tensor inputs must
                # share a base partition, and tg sits at base 64.
                m = wk.tile([128, B], f32, tag=f"m{hf}", name=f"m{hf}")
                u = wk.tile([64, B], f32, tag=f"u{hf}", name=f"u{hf}")
                nc.gpsimd.tensor_mul(m[64:128, :], ti, tg)
                nc.gpsimd.tensor_add(u[:], m[64:128, :], tg)
                # DVE: ca = (t_f + 1) * c2 ; c2' = 0.5*ca + u
                tf = T[0:64, 0:B]
                ca = wk.tile([64, B], f32, tag=f"ca{hf}", name=f"ca{hf}")
                nc.vector.scalar_tensor_tensor(ca[:], tf, 1.0, cst[hf][:],
                                               ALU.add, ALU.mult)
                nc.vector.scalar_tensor_tensor(cst[hf][:], ca[:], 0.5, u[:],
                                               ALU.mult, ALU.add)

            def phase_tanh_c(hf):
                tcv = wk.tile([64, B], f32, tag=f"tc{hf}", name=f"tc{hf}")
                nc.scalar.activation(tcv[:], cst[hf][:], AF.Tanh, scale=0.5)
                return tcv

            def phase_h(T, tcv, layer, s, hf):
                for di in (0, 1):
                    t_proc = s if di == 0 else s_steps - 1 - s
                    to_d = T[0:64, 2 * HB + di * HB:2 * HB + (di + 1) * HB]
                    tc_d = tcv[:, di * HB:(di + 1) * HB]
                    if layer == 0:
                        dst = h0_buf[di * 64:(di + 1) * 64,
                                     t_proc * B + hf * HB:t_proc * B + hf * HB + HB]
                        nc.vector.scalar_tensor_tensor(dst, to_d, 1.0, tc_d,
                                                       ALU.add, ALU.mult)
                    else:
                        dst = h1[hf][0:64, di * HB:(di + 1) * HB]
                        nc.vector.scalar_tensor_tensor(dst, to_d, 1.0, tc_d,
                                                       ALU.add, ALU.mult)
                        if (di == 0 and s == s_steps - 1) or (di == 1 and s == 0):
                            lst = h1_last[di * 64:(di + 1) * 64,
                                          hf * HB:hf * HB + HB]
                            nc.vector.scalar_tensor_tensor(lst, to_d, 1.0, tc_d,
                                                           ALU.add, ALU.mult)

            for layer in (0, 1):
                for hf in (0, 1):
                    nc.vector.memset(cst[hf][:], 0.0)
                    if layer == 1:
                        nc.vector.memset(h1[hf][0:64, :], 0.0)
                        nc.vector.memset(h1[hf][64:65, :], 1.0)
                for s in range(s_steps):
                    if layer == 0 and s % ct == 0:
                        k = s // ct + 2
                        if k < n_ch:
                            load_chunk("f", k)
                            load_chunk("r", k)
                    # half 0: matmuls + gates + cell update
                    G0 = phase_mm(layer, s, 0)
                    T0 = phase_gates(G0, 0)
                    phase_cup(T0, 0)
                    # half 1: matmuls + gates (fills ACT while DVE runs half 0)
                    G1 = phase_mm(layer, s, 1)
                    T1 = phase_gates(G1, 1)
                    tc0 = phase_tanh_c(0)
                    phase_cup(T1, 1)
                    phase_h(T0, tc0, layer, s, 0)
                    tc1 = phase_tanh_c(1)
                    phase_h(T1, tc1, layer, s, 1)

            # ---- fc head ----
            pfc = ps.tile([C_OUT, B], f32, tag="pfc", name="pfc")
            nc.tensor.matmul(pfc[:], fcT_sb[:], h1_last[:], start=True, stop=True)
            osb = wk.tile([C_OUT, B], f32, tag="osb", name="osb")
            nc.scalar.activation(osb[:], pfc[:], AF.Identity, bias=fcb_sb[:, 0:1])
            nc.sync.dma_start(out=out_d[:], in_=osb[:])

    _split_multi_waits(nc, mybir)
    return nc


_cached = {}


def kernel(**inputs):
    from concourse.bass_utils import run_bass_kernel_spmd

    if "nc" not in _cached:
        _cached["nc"] = build_nc(S)
    nc = _cached["nc"]

    shared = _host_prep(inputs)
    x = np.asarray(inputs["x"], np.float32)
    in_maps = []
    for c in range(NCORES):
        m = dict(shared)
        m["xT"] = _host_xT(x[c * B:(c + 1) * B])
        in_maps.append(m)

    res = run_bass_kernel_spmd(nc, in_maps, list(range(NCORES)))
    out = np.concatenate([r["out"].T for r in res.results], axis=0)
    return np.ascontiguousarray(out.astype(np.float32))


# revision 10
# speedup vs baseline: 547.2341x; 1.8103x over previous
"""Trainium2 Bass kernel for 2-layer bidirectional LSTM (B=1024,S=256,F=16,H=64).

Sharding: batch data-parallel across 8 cores (128 batch rows each), weights
replicated. Per core the 128-row batch is split into two 64-row halves that
run as staggered recurrence chains so engine work hides per-step dependency
latency.

All four gates of both directions are evaluated with a SINGLE Tanh ACT op per
half-step using sigma(a) = (1 + tanh(a/2)) / 2: the i/f/o gate rows of every
weight matrix and bias are pre-scaled by 1/2 on the host. Gate tiles are
gate-type-major and direction-stacked on partitions — the PSUM tile
[128, 256] has column blocks [F | I | O | G], each [dirF(64); dirR(64)] rows
— so every elementwise op in the cell update runs on full 128-partition
tiles of only 64 columns. With c stored as 2c and h stored as 2h:

    cb = (t_i + 1) * t_g            (DVE scalar_tensor_tensor)
    ca = (t_f + 1) * c2             (DVE scalar_tensor_tensor)
    c2' = 0.5 * ca + cb             (DVE scalar_tensor_tensor)
    tc = tanh(0.5 * c2')            (ACT, scale operand)
    h2 = (t_o + 1) * tc             (DVE scalar_tensor_tensor, fp16 out)

The 0.5 compensation for the doubled h is folded into every consumer weight
matrix on the host. Matmul inputs (x, h, weights) are fp16: 1 PE cycle/row
vs fp32's 4. Biases ride in the matmuls via a constant-one row appended to
the layer-0 x slab (K=17) and to the layer-1 h state tile (K=65), so the ACT
needs no bias operand. Each PSUM rectangle's ih (start) and hh (stop)
matmuls are emitted adjacently: interleaved open accumulation groups in one
bank lose the earlier contribution.
"""
import numpy as np

H = 64
B = 128          # batch per core
NH = 2           # number of staggered batch chains
HB = B // NH     # batch width per chain
S = 256
F = 16
CT = 32          # timesteps per streamed x chunk
NCORES = 8
FULL_B = 1024
C_OUT = 3

# gate blocks in kernel order f,i,o,g: (pytorch row offset, arg scale)
GATES = ((64, 0.5), (0, 0.5), (192, 0.5), (128, 1.0))


def _host_prep(inputs):
    """Shared (weight) DRAM arrays, replicated to every core.

    w0 [17, 512]  layer-0 ih lhsT: col block (gi*2+di)*64, bias in row 16
    u0 [128, 256] layer-0 hh lhsT: rows di*64:(di+1)*64, col block gi*64
    w1 [128, 512] layer-1 ih lhsT: col block (gi*2+di)*64
    u1 [65, 512]  layer-1 hh lhsT: col block (gi*2+di)*64, bias in row 64
    """
    w0 = np.zeros((F + 1, 512), np.float64)
    u0 = np.zeros((128, 256), np.float64)
    w1 = np.zeros((128, 512), np.float64)
    u1 = np.zeros((65, 512), np.float64)
    for layer in (0, 1):
        ihs = 1.0 if layer == 0 else 0.5
        for di, suf in enumerate(("", "r")):
            W = np.asarray(inputs[f"w_ih_l{layer}{suf}"], np.float64)
            U = np.asarray(inputs[f"w_hh_l{layer}{suf}"], np.float64)
            b = (np.asarray(inputs[f"b_ih_l{layer}{suf}"], np.float64)
                 + np.asarray(inputs[f"b_hh_l{layer}{suf}"], np.float64))
            for gi, (rlo, sc) in enumerate(GATES):
                wg = (ihs * sc * W[rlo:rlo + 64]).T      # [din, 64]
                ug = (0.5 * sc * U[rlo:rlo + 64]).T      # [64, 64]
                bg = sc * b[rlo:rlo + 64]                # [64]
                cb = (gi * 2 + di) * 64
                if layer == 0:
                    w0[0:F, cb:cb + 64] = wg
                    w0[F, cb:cb + 64] = bg
                    u0[di * 64:(di + 1) * 64, gi * 64:(gi + 1) * 64] = ug
                else:
                    w1[:, cb:cb + 64] = wg
                    u1[0:64, cb:cb + 64] = ug
                    u1[64, cb:cb + 64] = bg
    d = {"w0": w0.astype(np.float16), "u0": u0.astype(np.float16),
         "w1": w1.astype(np.float16), "u1": u1.astype(np.float16)}
    d["fcT"] = np.ascontiguousarray(
        (0.5 * np.asarray(inputs["fc_w"], np.float64)).T).astype(np.float16)
    d["fcb"] = np.asarray(inputs["fc_b"], np.float32).reshape(C_OUT, 1)
    return d


def _host_xT(x_core):
    """x_core [B, S, F] -> [F+1, S*B] fp16, col = t*B + b, last row = 1."""
    xt = np.asarray(x_core, np.float32).transpose(2, 1, 0).reshape(F, -1)
    ones = np.ones((1, xt.shape[1]), np.float32)
    return np.concatenate([xt, ones], axis=0).astype(np.float16)


def _patch_tile_drain():
    """This container's walrus rejects instructions carrying multiple sync
    waits ("Too many sync wait commands") — chunk the kernel-tail drain's
    global-clock waits into one drain instruction per semaphore."""
    import concourse.tile as tile
    from concourse.vector_clock import ScopedClock, VectorClock
    if getattr(tile.TileContext, "_drain_patched", False):
        return
    def patched_drain(self, tick_clock, wait_clock):
        gc = tick_clock.global_clock
        n = len(gc)
        procs = [i for i in range(n) if gc[i] > 0]
        chunks = [[p] for p in procs] or [[]]
        for ch in chunks:
            vec = [0] * n
            for p in ch:
                vec[p] = gc[p]
            d = self.nc.sync.drain()
            wait_clock.add_sem_waits(d.ins, ScopedClock({None: VectorClock(vec)}))
        self.nc.all_engine_barrier()
        popped = self.nc._tile_sem_poison_stack.pop()
        assert popped is self._sem_poison
        self.nc.clear_and_free_semaphores(list(self.sems.allocated().values()))
        self.nc.all_engine_barrier()
    tile.TileContext._drain_and_barrier = patched_drain
    tile.TileContext._drain_patched = True


def _split_multi_waits(nc, mybir):
    """This walrus build rejects instructions with more than one sync wait.
    Hoist extra waits onto same-engine NoOp instructions inserted immediately
    before the owning instruction (identical semantics: the engine is
    sequential, so waiting on the prior instruction slot is equivalent)."""
    for f in nc.m.functions:
        for bb in f.blocks:
            out = []
            changed = False
            for inst in bb.instructions:
                si = inst.sync_info
                waits = list(si.on_wait) if si is not None else []
                if len(waits) > 1:
                    changed = True
                    for w in waits[:-1]:
                        nop = mybir.InstNoOp(
                            name=nc.get_next_instruction_name(), ins=[], outs=[])
                        nop.engine = inst.engine
                        nop.sync_info = mybir.SyncInfo(on_wait=[w], on_update=[])
                        out.append(nop)
                    inst.sync_info = mybir.SyncInfo(
                        on_wait=[waits[-1]], on_update=list(si.on_update))
                out.append(inst)
            if changed:
                bb.instructions = out


def build_nc(s_steps=S):
    import concourse.bass as bass
    import concourse.tile as tile
    from concourse import mybir
    _patch_tile_drain()

    f32 = mybir.dt.float32
    f16 = mybir.dt.float16
    AF = mybir.ActivationFunctionType
    ALU = mybir.AluOpType

    ct = min(CT, s_steps)
    n_ch = s_steps // ct
    nc = bass.Bass("TRN2", target_bir_lowering=False, debug=False)

    xT_d = nc.dram_tensor("xT", [F + 1, s_steps * B], f16, kind="ExternalInput")
    wshape = {"w0": [F + 1, 512], "u0": [128, 256],
              "w1": [128, 512], "u1": [65, 512]}
    wd = {n: nc.dram_tensor(n, sh, f16, kind="ExternalInput")
          for n, sh in wshape.items()}
    fcT_d = nc.dram_tensor("fcT", [128, C_OUT], f16, kind="ExternalInput")
    fcb_d = nc.dram_tensor("fcb", [C_OUT, 1], f32, kind="ExternalInput")
    out_d = nc.dram_tensor("out", [C_OUT, B], f32, kind="ExternalOutput")

    with tile.TileContext(nc) as tc:
        with tc.tile_pool(name="pers", bufs=1) as pers, \
             tc.tile_pool(name="xch", bufs=3) as xch, \
             tc.tile_pool(name="wk", bufs=2) as wk, \
             tc.tile_pool(name="ps", bufs=2, space="PSUM") as ps, \
             tc.tile_pool(name="psf", bufs=1, space="PSUM") as psf:

            h0_buf = pers.tile([128, s_steps * B], f16, tag="h0buf", name="h0_buf")
            h1 = [pers.tile([65, 2 * HB], f16, tag=f"h1_{hf}", name=f"h1{hf}")
                  for hf in range(NH)]
            cst = [pers.tile([128, HB], f32, tag=f"c_{hf}", name=f"c{hf}")
                   for hf in range(NH)]
            h1_last = pers.tile([128, B], f16, tag="h1l", name="h1_last")

            wsb = {}
            for n, sh in wshape.items():
                t = pers.tile(sh, f16, tag=f"w_{n}", name=f"w_{n}")
                nc.sync.dma_start(out=t[:], in_=wd[n][:])
                wsb[n] = t
            fcT_sb = pers.tile([128, C_OUT], f16, tag="fcT", name="fcT_sb")
            nc.sync.dma_start(out=fcT_sb[:], in_=fcT_d[:])
            fcb_sb = pers.tile([C_OUT, 1], f32, tag="fcb", name="fcb_sb")
            nc.sync.dma_start(out=fcb_sb[:], in_=fcb_d[:])

            # --- x chunk streaming (layer 0 only), one stream per direction.
            chunks = {}

            def load_chunk(dc, k):
                t = xch.tile([F + 1, ct * B], f16, tag=f"x{dc}", name=f"x{dc}{k}")
                if dc == "f":
                    lo = k * ct * B
                else:
                    lo = (s_steps - (k + 1) * ct) * B
                nc.sync.dma_start(out=t[:], in_=xT_d[:, lo:lo + ct * B])
                chunks[dc, k] = t

            def x_rhs(dc, t_proc, hf):
                k = t_proc // ct if dc == "f" else (s_steps - 1 - t_proc) // ct
                ch = chunks[dc, k]
                base = k * ct if dc == "f" else s_steps - (k + 1) * ct
                off = (t_proc - base) * B + hf * HB
                return ch[:, off:off + HB]

            load_chunk("f", 0)
            load_chunk("r", 0)
            if n_ch > 1:
                load_chunk("f", 1)
                load_chunk("r", 1)

            def phase_mm(layer, s, hf):
                # Each rectangle's ih (start) matmul immediately followed by
                # its hh (stop) matmul — one open accumulation group at a
                # time per PSUM bank.
                G = ps.tile([128, 4 * HB], f32, tag=f"G{hf}", name=f"G{hf}_{layer}_{s}")
                first_step = s == 0
                for gi in range(4):
                    for di in (0, 1):
                        t_proc = s if di == 0 else s_steps - 1 - s
                        t_prev = s - 1 if di == 0 else s_steps - s
                        dst = G[di * 64:(di + 1) * 64, gi * HB:(gi + 1) * HB]
                        cb = (gi * 2 + di) * 64
                        if layer == 0:
                            rhs = x_rhs("f" if di == 0 else "r", t_proc, hf)
                            lhsT = wsb["w0"][:, cb:cb + 64]
                        else:
                            rhs = h0_buf[:, t_proc * B + hf * HB:
                                         t_proc * B + hf * HB + HB]
                            lhsT = wsb["w1"][:, cb:cb + 64]
                        only = layer == 0 and first_step
                        nc.tensor.matmul(dst, lhsT, rhs, start=True, stop=only)
                        if only:
                            continue
                        if layer == 0:
                            rhs2 = h0_buf[di * 64:(di + 1) * 64,
                                          t_prev * B + hf * HB:
                                          t_prev * B + hf * HB + HB]
                            lhsT2 = wsb["u0"][di * 64:(di + 1) * 64,
                                              gi * 64:(gi + 1) * 64]
                        else:
                            rhs2 = h1[hf][:, di * HB:(di + 1) * HB]
                            lhsT2 = wsb["u1"][:, cb:cb + 64]
                        nc.tensor.matmul(dst, lhsT2, rhs2, start=False, stop=True)
                return G

            def phase_gates(G, hf):
                T = wk.tile([128, 4 * HB], f32, tag=f"T{hf}", name=f"T{hf}")
                nc.scalar.activation(T[:], G[:], AF.Tanh)
                return T

            def phase_cup(T, hf):
                # col blocks: F 0:64, I 64:128, O 128:192, G 192:256
                cbt = wk.tile([128, HB], f32, tag=f"cb{hf}", name=f"cb{hf}")
                nc.vector.scalar_tensor_tensor(cbt[:], T[:, HB:2 * HB], 1.0,
                                               T[:, 3 * HB:4 * HB], ALU.add, ALU.mult)
                ca = wk.tile([128, HB], f32, tag=f"ca{hf}", name=f"ca{hf}")
                nc.vector.scalar_tensor_tensor(ca[:], T[:, 0:HB], 1.0,
                                               cst[hf][:], ALU.add, ALU.mult)
                nc.vector.scalar_tensor_tensor(cst[hf][:], ca[:], 0.5, cbt[:],
                                               ALU.mult, ALU.add)

            def phase_tanh_c(hf):
                tcv = wk.tile([128, HB], f32, tag=f"tc{hf}", name=f"tc{hf}")
                nc.scalar.activation(tcv[:], cst[hf][:], AF.Tanh, scale=0.5)
                return tcv

            def phase_h(T, tcv, layer, s, hf):
                for di in (0, 1):
                    t_proc = s if di == 0 else s_steps - 1 - s
                    to_d = T[di * 64:(di + 1) * 64, 2 * HB:3 * HB]
                    tc_d = tcv[di * 64:(di + 1) * 64, :]
                    if layer == 0:
                        dst = h0_buf[di * 64:(di + 1) * 64,
                                     t_proc * B + hf * HB:t_proc * B + hf * HB + HB]
                        nc.vector.scalar_tensor_tensor(dst, to_d, 1.0, tc_d,
                                                       ALU.add, ALU.mult)
                    else:
                        dst = h1[hf][0:64, di * HB:(di + 1) * HB]
                        nc.vector.scalar_tensor_tensor(dst, to_d, 1.0, tc_d,
                                                       ALU.add, ALU.mult)
                        if (di == 0 and s == s_steps - 1) or (di == 1 and s == 0):
                            lst = h1_last[di * 64:(di + 1) * 64,
                                          hf * HB:hf * HB + HB]
                            nc.vector.scalar_tensor_tensor(lst, to_d, 1.0, tc_d,
                                                           ALU.add, ALU.mult)

            for layer in (0, 1):
                for hf in range(NH):
                    nc.vector.memset(cst[hf][:], 0.0)
                    if layer == 1:
                        nc.vector.memset(h1[hf][0:64, :], 0.0)
                        nc.vector.memset(h1[hf][64:65, :], 1.0)
                for s in range(s_steps):
                    if layer == 0 and s % ct == 0:
                        k = s // ct + 2
                        if k < n_ch:
                            load_chunk("f", k)
                            load_chunk("r", k)
                    Ts = [None] * NH
                    for hf in range(NH):
                        G = phase_mm(layer, s, hf)
                        Ts[hf] = phase_gates(G, hf)
                        if hf >= 1:
                            tcp = phase_tanh_c(hf - 1)
                        phase_cup(Ts[hf], hf)
                        if hf >= 1:
                            phase_h(Ts[hf - 1], tcp, layer, s, hf - 1)
                    tcl = phase_tanh_c(NH - 1)
                    phase_h(Ts[NH - 1], tcl, layer, s, NH - 1)

            # ---- fc head ----
            pfc = psf.tile([C_OUT, B], f32, tag="pfc", name="pfc")
            nc.tensor.matmul(pfc[:], fcT_sb[:], h1_last[:], start=True, stop=True)
            osb = wk.tile([C_OUT, B], f32, tag="osb", name="osb")
            nc.scalar.activation(osb[:], pfc[:], AF.Identity, bias=fcb_sb[:, 0:1])
            nc.sync.dma_start(out=out_d[:], in_=osb[:])

    _split_multi_waits(nc, mybir)
    return nc


_cached = {}


def kernel(**inputs):
    from concourse.bass_utils import run_bass_kernel_spmd

    if "nc" not in _cached:
        _cached["nc"] = build_nc(S)
    nc = _cached["nc"]

    shared = _host_prep(inputs)
    x = np.asarray(inputs["x"], np.float32)
    in_maps = []
    for c in range(NCORES):
        m = dict(shared)
        m["xT"] = _host_xT(x[c * B:(c + 1) * B])
        in_maps.append(m)

    res = run_bass_kernel_spmd(nc, in_maps, list(range(NCORES)))
    out = np.concatenate([r["out"].T for r in res.results], axis=0)
    return np.ascontiguousarray(out.astype(np.float32))


# revision 11
# speedup vs baseline: 550.4883x; 1.0059x over previous
"""Trainium2 Bass kernel for 2-layer bidirectional LSTM (B=1024,S=256,F=16,H=64).

Sharding: batch data-parallel across 8 cores (128 batch rows each), weights
replicated. Per core the 128-row batch is split into two 64-row halves that
run as staggered recurrence chains so engine work hides per-step dependency
latency.

All four gates of both directions are evaluated with a SINGLE Tanh ACT op per
half-step using sigma(a) = (1 + tanh(a/2)) / 2: the i/f/o gate rows of every
weight matrix and bias are pre-scaled by 1/2 on the host. Gate tiles are
gate-type-major and direction-stacked on partitions — the PSUM tile
[128, 256] has column blocks [F | I | O | G], each [dirF(64); dirR(64)] rows
— so every elementwise op in the cell update runs on full 128-partition
tiles of only 64 columns. With c stored as 2c and h stored as 2h:

    cb = (t_i + 1) * t_g            (DVE scalar_tensor_tensor)
    ca = (t_f + 1) * c2             (DVE scalar_tensor_tensor)
    c2' = 0.5 * ca + cb             (DVE scalar_tensor_tensor)
    tc = tanh(0.5 * c2')            (ACT, scale operand)
    h2 = (t_o + 1) * tc             (DVE scalar_tensor_tensor, fp16 out)

The 0.5 compensation for the doubled h is folded into every consumer weight
matrix on the host. Matmul inputs (x, h, weights) are fp16: 1 PE cycle/row
vs fp32's 4. Biases ride in the matmuls via a constant-one row appended to
the layer-0 x slab (K=17) and to the layer-1 h state tile (K=65), so the ACT
needs no bias operand. Each PSUM rectangle's ih (start) and hh (stop)
matmuls are emitted adjacently: interleaved open accumulation groups in one
bank lose the earlier contribution.
"""
import numpy as np

H = 64
B = 128          # batch per core
NH = 2           # number of staggered batch chains
HB = B // NH     # batch width per chain
S = 256
F = 16
CT = 32          # timesteps per streamed x chunk
NCORES = 8
FULL_B = 1024
C_OUT = 3

# gate blocks in kernel order f,i,o,g: (pytorch row offset, arg scale)
GATES = ((64, 0.5), (0, 0.5), (192, 0.5), (128, 1.0))


def _host_prep(inputs):
    """Shared (weight) DRAM arrays, replicated to every core.

    w0 [17, 512]  layer-0 ih lhsT: col block (gi*2+di)*64, bias in row 16
    u0 [128, 256] layer-0 hh lhsT: rows di*64:(di+1)*64, col block gi*64
    w1 [128, 512] layer-1 ih lhsT: col block (gi*2+di)*64
    u1 [65, 512]  layer-1 hh lhsT: col block (gi*2+di)*64, bias in row 64
    """
    w0 = np.zeros((F + 1, 512), np.float64)
    u0 = np.zeros((128, 256), np.float64)
    w1 = np.zeros((128, 512), np.float64)
    u1 = np.zeros((65, 512), np.float64)
    for layer in (0, 1):
        ihs = 1.0 if layer == 0 else 0.5
        for di, suf in enumerate(("", "r")):
            W = np.asarray(inputs[f"w_ih_l{layer}{suf}"], np.float64)
            U = np.asarray(inputs[f"w_hh_l{layer}{suf}"], np.float64)
            b = (np.asarray(inputs[f"b_ih_l{layer}{suf}"], np.float64)
                 + np.asarray(inputs[f"b_hh_l{layer}{suf}"], np.float64))
            for gi, (rlo, sc) in enumerate(GATES):
                wg = (ihs * sc * W[rlo:rlo + 64]).T      # [din, 64]
                ug = (0.5 * sc * U[rlo:rlo + 64]).T      # [64, 64]
                bg = sc * b[rlo:rlo + 64]                # [64]
                cb = (gi * 2 + di) * 64
                if layer == 0:
                    w0[0:F, cb:cb + 64] = wg
                    w0[F, cb:cb + 64] = bg
                    u0[di * 64:(di + 1) * 64, gi * 64:(gi + 1) * 64] = ug
                else:
                    w1[:, cb:cb + 64] = wg
                    u1[0:64, cb:cb + 64] = ug
                    u1[64, cb:cb + 64] = bg
    d = {"w0": w0.astype(np.float16), "u0": u0.astype(np.float16),
         "w1": w1.astype(np.float16), "u1": u1.astype(np.float16)}
    d["fcT"] = np.ascontiguousarray(
        (0.5 * np.asarray(inputs["fc_w"], np.float64)).T).astype(np.float16)
    d["fcb"] = np.asarray(inputs["fc_b"], np.float32).reshape(C_OUT, 1)
    return d


def _host_xT(x_core):
    """x_core [B, S, F] -> [F+1, S*B] fp16, col = t*B + b, last row = 1."""
    xt = np.asarray(x_core, np.float32).transpose(2, 1, 0).reshape(F, -1)
    ones = np.ones((1, xt.shape[1]), np.float32)
    return np.concatenate([xt, ones], axis=0).astype(np.float16)


def _patch_tile_drain():
    """This container's walrus rejects instructions carrying multiple sync
    waits ("Too many sync wait commands") — chunk the kernel-tail drain's
    global-clock waits into one drain instruction per semaphore."""
    import concourse.tile as tile
    from concourse.vector_clock import ScopedClock, VectorClock
    if getattr(tile.TileContext, "_drain_patched", False):
        return
    def patched_drain(self, tick_clock, wait_clock):
        gc = tick_clock.global_clock
        n = len(gc)
        procs = [i for i in range(n) if gc[i] > 0]
        chunks = [[p] for p in procs] or [[]]
        for ch in chunks:
            vec = [0] * n
            for p in ch:
                vec[p] = gc[p]
            d = self.nc.sync.drain()
            wait_clock.add_sem_waits(d.ins, ScopedClock({None: VectorClock(vec)}))
        self.nc.all_engine_barrier()
        popped = self.nc._tile_sem_poison_stack.pop()
        assert popped is self._sem_poison
        self.nc.clear_and_free_semaphores(list(self.sems.allocated().values()))
        self.nc.all_engine_barrier()
    tile.TileContext._drain_and_barrier = patched_drain
    tile.TileContext._drain_patched = True


def _split_multi_waits(nc, mybir):
    """This walrus build rejects instructions with more than one sync wait.
    Hoist extra waits onto same-engine NoOp instructions inserted immediately
    before the owning instruction (identical semantics: the engine is
    sequential, so waiting on the prior instruction slot is equivalent)."""
    for f in nc.m.functions:
        for bb in f.blocks:
            out = []
            changed = False
            for inst in bb.instructions:
                si = inst.sync_info
                waits = list(si.on_wait) if si is not None else []
                if len(waits) > 1:
                    changed = True
                    for w in waits[:-1]:
                        nop = mybir.InstNoOp(
                            name=nc.get_next_instruction_name(), ins=[], outs=[])
                        nop.engine = inst.engine
                        nop.sync_info = mybir.SyncInfo(on_wait=[w], on_update=[])
                        out.append(nop)
                    inst.sync_info = mybir.SyncInfo(
                        on_wait=[waits[-1]], on_update=list(si.on_update))
                out.append(inst)
            if changed:
                bb.instructions = out


def build_nc(s_steps=S):
    import concourse.bass as bass
    import concourse.tile as tile
    from concourse import mybir
    _patch_tile_drain()

    f32 = mybir.dt.float32
    f16 = mybir.dt.float16
    AF = mybir.ActivationFunctionType
    ALU = mybir.AluOpType

    ct = min(CT, s_steps)
    n_ch = s_steps // ct
    nc = bass.Bass("TRN2", target_bir_lowering=False, debug=False)

    xT_d = nc.dram_tensor("xT", [F + 1, s_steps * B], f16, kind="ExternalInput")
    wshape = {"w0": [F + 1, 512], "u0": [128, 256],
              "w1": [128, 512], "u1": [65, 512]}
    wd = {n: nc.dram_tensor(n, sh, f16, kind="ExternalInput")
          for n, sh in wshape.items()}
    fcT_d = nc.dram_tensor("fcT", [128, C_OUT], f16, kind="ExternalInput")
    fcb_d = nc.dram_tensor("fcb", [C_OUT, 1], f32, kind="ExternalInput")
    out_d = nc.dram_tensor("out", [C_OUT, B], f32, kind="ExternalOutput")

    with tile.TileContext(nc) as tc:
        with tc.tile_pool(name="pers", bufs=1) as pers, \
             tc.tile_pool(name="xch", bufs=3) as xch, \
             tc.tile_pool(name="wk", bufs=2) as wk, \
             tc.tile_pool(name="ps", bufs=2, space="PSUM") as ps, \
             tc.tile_pool(name="psf", bufs=1, space="PSUM") as psf:

            h0_buf = pers.tile([128, s_steps * B], f16, tag="h0buf", name="h0_buf")
            h1 = [pers.tile([65, 2 * HB], f16, tag=f"h1_{hf}", name=f"h1{hf}")
                  for hf in range(NH)]
            cst = [pers.tile([128, HB], f32, tag=f"c_{hf}", name=f"c{hf}")
                   for hf in range(NH)]
            h1_last = pers.tile([128, B], f16, tag="h1l", name="h1_last")

            wsb = {}
            for n, sh in wshape.items():
                t = pers.tile(sh, f16, tag=f"w_{n}", name=f"w_{n}")
                nc.sync.dma_start(out=t[:], in_=wd[n][:])
                wsb[n] = t
            fcT_sb = pers.tile([128, C_OUT], f16, tag="fcT", name="fcT_sb")
            nc.sync.dma_start(out=fcT_sb[:], in_=fcT_d[:])
            fcb_sb = pers.tile([C_OUT, 1], f32, tag="fcb", name="fcb_sb")
            nc.sync.dma_start(out=fcb_sb[:], in_=fcb_d[:])

            # --- x chunk streaming (layer 0 only), one stream per direction.
            chunks = {}

            def load_chunk(dc, k):
                t = xch.tile([F + 1, ct * B], f16, tag=f"x{dc}", name=f"x{dc}{k}")
                if dc == "f":
                    lo = k * ct * B
                else:
                    lo = (s_steps - (k + 1) * ct) * B
                nc.sync.dma_start(out=t[:], in_=xT_d[:, lo:lo + ct * B])
                chunks[dc, k] = t

            def x_rhs(dc, t_proc, hf):
                k = t_proc // ct if dc == "f" else (s_steps - 1 - t_proc) // ct
                ch = chunks[dc, k]
                base = k * ct if dc == "f" else s_steps - (k + 1) * ct
                off = (t_proc - base) * B + hf * HB
                return ch[:, off:off + HB]

            load_chunk("f", 0)
            load_chunk("r", 0)
            if n_ch > 1:
                load_chunk("f", 1)
                load_chunk("r", 1)

            def phase_mm(layer, s, hf):
                # Each rectangle's ih (start) matmul immediately followed by
                # its hh (stop) matmul — one open accumulation group at a
                # time per PSUM bank.
                G = ps.tile([128, 4 * HB], f32, tag=f"G{hf}", name=f"G{hf}_{layer}_{s}")
                first_step = s == 0
                for di in (0, 1):
                    for gi in range(4):
                        t_proc = s if di == 0 else s_steps - 1 - s
                        t_prev = s - 1 if di == 0 else s_steps - s
                        dst = G[di * 64:(di + 1) * 64, gi * HB:(gi + 1) * HB]
                        cb = (gi * 2 + di) * 64
                        if layer == 0:
                            rhs = x_rhs("f" if di == 0 else "r", t_proc, hf)
                            lhsT = wsb["w0"][:, cb:cb + 64]
                        else:
                            rhs = h0_buf[:, t_proc * B + hf * HB:
                                         t_proc * B + hf * HB + HB]
                            lhsT = wsb["w1"][:, cb:cb + 64]
                        only = layer == 0 and first_step
                        nc.tensor.matmul(dst, lhsT, rhs, start=True, stop=only)
                        if only:
                            continue
                        if layer == 0:
                            rhs2 = h0_buf[di * 64:(di + 1) * 64,
                                          t_prev * B + hf * HB:
                                          t_prev * B + hf * HB + HB]
                            lhsT2 = wsb["u0"][di * 64:(di + 1) * 64,
                                              gi * 64:(gi + 1) * 64]
                        else:
                            rhs2 = h1[hf][:, di * HB:(di + 1) * HB]
                            lhsT2 = wsb["u1"][:, cb:cb + 64]
                        nc.tensor.matmul(dst, lhsT2, rhs2, start=False, stop=True)
                return G

            def phase_gates(G, hf):
                T = wk.tile([128, 4 * HB], f32, tag=f"T{hf}", name=f"T{hf}")
                nc.scalar.activation(T[:], G[:], AF.Tanh)
                return T

            def phase_cup(T, hf):
                # col blocks: F 0:64, I 64:128, O 128:192, G 192:256
                cbt = wk.tile([128, HB], f32, tag=f"cb{hf}", name=f"cb{hf}")
                nc.vector.scalar_tensor_tensor(cbt[:], T[:, HB:2 * HB], 1.0,
                                               T[:, 3 * HB:4 * HB], ALU.add, ALU.mult)
                ca = wk.tile([128, HB], f32, tag=f"ca{hf}", name=f"ca{hf}")
                nc.vector.scalar_tensor_tensor(ca[:], T[:, 0:HB], 1.0,
                                               cst[hf][:], ALU.add, ALU.mult)
                nc.vector.scalar_tensor_tensor(cst[hf][:], ca[:], 0.5, cbt[:],
                                               ALU.mult, ALU.add)

            def phase_tanh_c(hf):
                tcv = wk.tile([128, HB], f32, tag=f"tc{hf}", name=f"tc{hf}")
                nc.scalar.activation(tcv[:], cst[hf][:], AF.Tanh, scale=0.5)
                return tcv

            def phase_h(T, tcv, layer, s, hf):
                for di in (0, 1):
                    t_proc = s if di == 0 else s_steps - 1 - s
                    to_d = T[di * 64:(di + 1) * 64, 2 * HB:3 * HB]
                    tc_d = tcv[di * 64:(di + 1) * 64, :]
                    if layer == 0:
                        dst = h0_buf[di * 64:(di + 1) * 64,
                                     t_proc * B + hf * HB:t_proc * B + hf * HB + HB]
                        nc.vector.scalar_tensor_tensor(dst, to_d, 1.0, tc_d,
                                                       ALU.add, ALU.mult)
                    else:
                        dst = h1[hf][0:64, di * HB:(di + 1) * HB]
                        nc.vector.scalar_tensor_tensor(dst, to_d, 1.0, tc_d,
                                                       ALU.add, ALU.mult)
                        if (di == 0 and s == s_steps - 1) or (di == 1 and s == 0):
                            lst = h1_last[di * 64:(di + 1) * 64,
                                          hf * HB:hf * HB + HB]
                            nc.vector.scalar_tensor_tensor(lst, to_d, 1.0, tc_d,
                                                           ALU.add, ALU.mult)

            for layer in (0, 1):
                for hf in range(NH):
                    nc.vector.memset(cst[hf][:], 0.0)
                    if layer == 1:
                        nc.vector.memset(h1[hf][0:64, :], 0.0)
                        nc.vector.memset(h1[hf][64:65, :], 1.0)
                for s in range(s_steps):
                    if layer == 0 and s % ct == 0:
                        k = s // ct + 2
                        if k < n_ch:
                            load_chunk("f", k)
                            load_chunk("r", k)
                    Ts = [None] * NH
                    for hf in range(NH):
                        G = phase_mm(layer, s, hf)
                        Ts[hf] = phase_gates(G, hf)
                        if hf >= 1:
                            tcp = phase_tanh_c(hf - 1)
                        phase_cup(Ts[hf], hf)
                        if hf >= 1:
                            phase_h(Ts[hf - 1], tcp, layer, s, hf - 1)
                    tcl = phase_tanh_c(NH - 1)
                    phase_h(Ts[NH - 1], tcl, layer, s, NH - 1)

            # ---- fc head ----
            pfc = psf.tile([C_OUT, B], f32, tag="pfc", name="pfc")
            nc.tensor.matmul(pfc[:], fcT_sb[:], h1_last[:], start=True, stop=True)
            osb = wk.tile([C_OUT, B], f32, tag="osb", name="osb")
            nc.scalar.activation(osb[:], pfc[:], AF.Identity, bias=fcb_sb[:, 0:1])
            nc.sync.dma_start(out=out_d[:], in_=osb[:])

    _split_multi_waits(nc, mybir)
    return nc


_cached = {}


def kernel(**inputs):
    from concourse.bass_utils import run_bass_kernel_spmd

    if "nc" not in _cached:
        _cached["nc"] = build_nc(S)
    nc = _cached["nc"]

    shared = _host_prep(inputs)
    x = np.asarray(inputs["x"], np.float32)
    in_maps = []
    for c in range(NCORES):
        m = dict(shared)
        m["xT"] = _host_xT(x[c * B:(c + 1) * B])
        in_maps.append(m)

    res = run_bass_kernel_spmd(nc, in_maps, list(range(NCORES)))
    out = np.concatenate([r["out"].T for r in res.results], axis=0)
    return np.ascontiguousarray(out.astype(np.float32))


# revision 14
# speedup vs baseline: 579.7920x; 1.0532x over previous
"""Trainium2 Bass kernel for 2-layer bidirectional LSTM (B=1024,S=256,F=16,H=64).

Sharding: batch data-parallel across 8 cores (128 batch rows each), weights
replicated. Per core the 128-row batch is split into two 64-row halves that
run as staggered recurrence chains so engine work hides per-step dependency
latency.

All four gates of both directions are evaluated with a SINGLE Tanh ACT op per
half-step using sigma(a) = (1 + tanh(a/2)) / 2: the i/f/o gate rows of every
weight matrix and bias are pre-scaled by 1/2 on the host. Gate tiles are
gate-type-major and direction-stacked on partitions — the PSUM tile
[128, 256] has column blocks [F | I | O | G], each [dirF(64); dirR(64)] rows
— so every elementwise op in the cell update runs on full 128-partition
tiles of only 64 columns. With c stored as 2c and h stored as 2h:

    cb = (t_i + 1) * t_g            (DVE scalar_tensor_tensor)
    ca = (t_f + 1) * c2             (DVE scalar_tensor_tensor)
    c2' = 0.5 * ca + cb             (DVE scalar_tensor_tensor)
    tc = tanh(0.5 * c2')            (ACT, scale operand)
    h2 = (t_o + 1) * tc             (DVE scalar_tensor_tensor, fp16 out)

The 0.5 compensation for the doubled h is folded into every consumer weight
matrix on the host. Matmul inputs (x, h, weights) are fp16: 1 PE cycle/row
vs fp32's 4. Biases ride in the matmuls via a constant-one row appended to
the layer-0 x slab (K=17) and to the layer-1 h state tile (K=65), so the ACT
needs no bias operand. Each PSUM rectangle's ih (start) and hh (stop)
matmuls are emitted adjacently: interleaved open accumulation groups in one
bank lose the earlier contribution.
"""
import numpy as np

H = 64
B = 128          # batch per core
NH = 2           # number of staggered batch chains
HB = B // NH     # batch width per chain
S = 256
F = 16
CT = 32          # timesteps per streamed x chunk
NCORES = 8
FULL_B = 1024
C_OUT = 3

# gate blocks in kernel order f,i,o,g: (pytorch row offset, arg scale)
GATES = ((64, 0.5), (0, 0.5), (192, 0.5), (128, 1.0))


def _host_prep(inputs):
    """Shared (weight) DRAM arrays, replicated to every core.

    w0 [17, 512]  layer-0 ih lhsT: col block (gi*2+di)*64, bias in row 16
    u0 [128, 256] layer-0 hh lhsT: rows di*64:(di+1)*64, col block gi*64
    w1 [128, 512] layer-1 ih lhsT: col block (gi*2+di)*64
    u1 [65, 512]  layer-1 hh lhsT: col block (gi*2+di)*64, bias in row 64
    """
    w0 = np.zeros((F + 1, 512), np.float64)
    u0 = np.zeros((128, 256), np.float64)
    w1 = np.zeros((128, 512), np.float64)
    u1 = np.zeros((65, 512), np.float64)
    for layer in (0, 1):
        ihs = 1.0 if layer == 0 else 0.5
        for di, suf in enumerate(("", "r")):
            W = np.asarray(inputs[f"w_ih_l{layer}{suf}"], np.float64)
            U = np.asarray(inputs[f"w_hh_l{layer}{suf}"], np.float64)
            b = (np.asarray(inputs[f"b_ih_l{layer}{suf}"], np.float64)
                 + np.asarray(inputs[f"b_hh_l{layer}{suf}"], np.float64))
            for gi, (rlo, sc) in enumerate(GATES):
                wg = (ihs * sc * W[rlo:rlo + 64]).T      # [din, 64]
                ug = (0.5 * sc * U[rlo:rlo + 64]).T      # [64, 64]
                bg = sc * b[rlo:rlo + 64]                # [64]
                cb = (gi * 2 + di) * 64
                if layer == 0:
                    w0[0:F, cb:cb + 64] = wg
                    w0[F, cb:cb + 64] = bg
                    u0[di * 64:(di + 1) * 64, gi * 64:(gi + 1) * 64] = ug
                else:
                    w1[:, cb:cb + 64] = wg
                    u1[0:64, cb:cb + 64] = ug
                    u1[64, cb:cb + 64] = bg
    d = {"w0": w0.astype(np.float16), "u0": u0.astype(np.float16),
         "w1": w1.astype(np.float16), "u1": u1.astype(np.float16)}
    d["fcT"] = np.ascontiguousarray(
        (0.5 * np.asarray(inputs["fc_w"], np.float64)).T).astype(np.float16)
    d["fcb"] = np.asarray(inputs["fc_b"], np.float32).reshape(C_OUT, 1)
    return d


def _host_xT(x_core):
    """x_core [B, S, F] -> [F+1, S*B] fp16, col = t*B + b, last row = 1."""
    xt = np.asarray(x_core, np.float32).transpose(2, 1, 0).reshape(F, -1)
    ones = np.ones((1, xt.shape[1]), np.float32)
    return np.concatenate([xt, ones], axis=0).astype(np.float16)


def _patch_tile_drain():
    """This container's walrus rejects instructions carrying multiple sync
    waits ("Too many sync wait commands") — chunk the kernel-tail drain's
    global-clock waits into one drain instruction per semaphore."""
    import concourse.tile as tile
    from concourse.vector_clock import ScopedClock, VectorClock
    if getattr(tile.TileContext, "_drain_patched", False):
        return
    def patched_drain(self, tick_clock, wait_clock):
        gc = tick_clock.global_clock
        n = len(gc)
        procs = [i for i in range(n) if gc[i] > 0]
        chunks = [[p] for p in procs] or [[]]
        for ch in chunks:
            vec = [0] * n
            for p in ch:
                vec[p] = gc[p]
            d = self.nc.sync.drain()
            wait_clock.add_sem_waits(d.ins, ScopedClock({None: VectorClock(vec)}))
        self.nc.all_engine_barrier()
        popped = self.nc._tile_sem_poison_stack.pop()
        assert popped is self._sem_poison
        self.nc.clear_and_free_semaphores(list(self.sems.allocated().values()))
        self.nc.all_engine_barrier()
    tile.TileContext._drain_and_barrier = patched_drain
    tile.TileContext._drain_patched = True


def _split_multi_waits(nc, mybir):
    """This walrus build rejects instructions with more than one sync wait.
    Hoist extra waits onto same-engine NoOp instructions inserted immediately
    before the owning instruction (identical semantics: the engine is
    sequential, so waiting on the prior instruction slot is equivalent)."""
    for f in nc.m.functions:
        for bb in f.blocks:
            out = []
            changed = False
            for inst in bb.instructions:
                si = inst.sync_info
                waits = list(si.on_wait) if si is not None else []
                if len(waits) > 1:
                    changed = True
                    for w in waits[:-1]:
                        nop = mybir.InstNoOp(
                            name=nc.get_next_instruction_name(), ins=[], outs=[])
                        nop.engine = inst.engine
                        nop.sync_info = mybir.SyncInfo(on_wait=[w], on_update=[])
                        out.append(nop)
                    inst.sync_info = mybir.SyncInfo(
                        on_wait=[waits[-1]], on_update=list(si.on_update))
                out.append(inst)
            if changed:
                bb.instructions = out


def build_nc(s_steps=S, reps=1):
    """reps > 1 unrolls the whole computation `reps` times inside one NEFF
    (same tiles, same output), letting the timing harness amortize the
    per-execution runtime launch overhead; the result is identical."""
    import concourse.bass as bass
    import concourse.tile as tile
    from concourse import mybir
    _patch_tile_drain()

    f32 = mybir.dt.float32
    f16 = mybir.dt.float16
    AF = mybir.ActivationFunctionType
    ALU = mybir.AluOpType

    ct = min(CT, s_steps)
    n_ch = s_steps // ct
    nc = bass.Bass("TRN2", target_bir_lowering=False, debug=False)

    xT_d = nc.dram_tensor("xT", [F + 1, s_steps * B], f16, kind="ExternalInput")
    wshape = {"w0": [F + 1, 512], "u0": [128, 256],
              "w1": [128, 512], "u1": [65, 512]}
    wd = {n: nc.dram_tensor(n, sh, f16, kind="ExternalInput")
          for n, sh in wshape.items()}
    fcT_d = nc.dram_tensor("fcT", [128, C_OUT], f16, kind="ExternalInput")
    fcb_d = nc.dram_tensor("fcb", [C_OUT, 1], f32, kind="ExternalInput")
    out_d = nc.dram_tensor("out", [C_OUT, B], f32, kind="ExternalOutput")

    with tile.TileContext(nc) as tc:
        with tc.tile_pool(name="pers", bufs=1) as pers, \
             tc.tile_pool(name="xch", bufs=3) as xch, \
             tc.tile_pool(name="wk", bufs=2) as wk, \
             tc.tile_pool(name="ps", bufs=2, space="PSUM") as ps, \
             tc.tile_pool(name="psf", bufs=1, space="PSUM") as psf:

            h0_buf = pers.tile([128, s_steps * B], f16, tag="h0buf", name="h0_buf")
            h1 = [pers.tile([65, 2 * HB], f16, tag=f"h1_{hf}", name=f"h1{hf}")
                  for hf in range(NH)]
            cst = [pers.tile([128, HB], f32, tag=f"c_{hf}", name=f"c{hf}")
                   for hf in range(NH)]
            h1_last = pers.tile([128, B], f16, tag="h1l", name="h1_last")

            wsb = {}
            for n, sh in wshape.items():
                t = pers.tile(sh, f16, tag=f"w_{n}", name=f"w_{n}")
                nc.sync.dma_start(out=t[:], in_=wd[n][:])
                wsb[n] = t
            fcT_sb = pers.tile([128, C_OUT], f16, tag="fcT", name="fcT_sb")
            nc.sync.dma_start(out=fcT_sb[:], in_=fcT_d[:])
            fcb_sb = pers.tile([C_OUT, 1], f32, tag="fcb", name="fcb_sb")
            nc.sync.dma_start(out=fcb_sb[:], in_=fcb_d[:])

            # --- x chunk streaming (layer 0 only), one stream per direction.
            chunks = {}

            def load_chunk(dc, k):
                t = xch.tile([F + 1, ct * B], f16, tag=f"x{dc}", name=f"x{dc}{k}")
                if dc == "f":
                    lo = k * ct * B
                else:
                    lo = (s_steps - (k + 1) * ct) * B
                nc.sync.dma_start(out=t[:], in_=xT_d[:, lo:lo + ct * B])
                chunks[dc, k] = t

            def x_rhs(dc, t_proc, hf):
                k = t_proc // ct if dc == "f" else (s_steps - 1 - t_proc) // ct
                ch = chunks[dc, k]
                base = k * ct if dc == "f" else s_steps - (k + 1) * ct
                off = (t_proc - base) * B + hf * HB
                return ch[:, off:off + HB]

            def phase_mm(layer, s, hf):
                # Each rectangle's ih (start) matmul immediately followed by
                # its hh (stop) matmul — one open accumulation group at a
                # time per PSUM bank.
                G = ps.tile([128, 4 * HB], f32, tag=f"G{hf}", name=f"G{hf}_{layer}_{s}")
                first_step = s == 0
                for di in (0, 1):
                    for gi in range(4):
                        t_proc = s if di == 0 else s_steps - 1 - s
                        t_prev = s - 1 if di == 0 else s_steps - s
                        dst = G[di * 64:(di + 1) * 64, gi * HB:(gi + 1) * HB]
                        cb = (gi * 2 + di) * 64
                        if layer == 0:
                            rhs = x_rhs("f" if di == 0 else "r", t_proc, hf)
                            lhsT = wsb["w0"][:, cb:cb + 64]
                        else:
                            rhs = h0_buf[:, t_proc * B + hf * HB:
                                         t_proc * B + hf * HB + HB]
                            lhsT = wsb["w1"][:, cb:cb + 64]
                        only = layer == 0 and first_step
                        nc.tensor.matmul(dst, lhsT, rhs, start=True, stop=only)
                        if only:
                            continue
                        if layer == 0:
                            rhs2 = h0_buf[di * 64:(di + 1) * 64,
                                          t_prev * B + hf * HB:
                                          t_prev * B + hf * HB + HB]
                            lhsT2 = wsb["u0"][di * 64:(di + 1) * 64,
                                              gi * 64:(gi + 1) * 64]
                        else:
                            rhs2 = h1[hf][:, di * HB:(di + 1) * HB]
                            lhsT2 = wsb["u1"][:, cb:cb + 64]
                        nc.tensor.matmul(dst, lhsT2, rhs2, start=False, stop=True)
                return G

            def phase_gates(G, hf):
                T = wk.tile([128, 4 * HB], f32, tag=f"T{hf}", name=f"T{hf}")
                nc.scalar.activation(T[:], G[:], AF.Tanh)
                return T

            def phase_cup(T, hf):
                # col blocks: F 0:64, I 64:128, O 128:192, G 192:256
                cbt = wk.tile([128, HB], f32, tag=f"cb{hf}", name=f"cb{hf}")
                nc.vector.scalar_tensor_tensor(cbt[:], T[:, HB:2 * HB], 1.0,
                                               T[:, 3 * HB:4 * HB], ALU.add, ALU.mult)
                ca = wk.tile([128, HB], f32, tag=f"ca{hf}", name=f"ca{hf}")
                nc.vector.scalar_tensor_tensor(ca[:], T[:, 0:HB], 1.0,
                                               cst[hf][:], ALU.add, ALU.mult)
                nc.vector.scalar_tensor_tensor(cst[hf][:], ca[:], 0.5, cbt[:],
                                               ALU.mult, ALU.add)

            def phase_tanh_c(hf):
                tcv = wk.tile([128, HB], f32, tag=f"tc{hf}", name=f"tc{hf}")
                nc.scalar.activation(tcv[:], cst[hf][:], AF.Tanh, scale=0.5)
                return tcv

            def phase_h(T, tcv, layer, s, hf):
                for di in (0, 1):
                    t_proc = s if di == 0 else s_steps - 1 - s
                    to_d = T[di * 64:(di + 1) * 64, 2 * HB:3 * HB]
                    tc_d = tcv[di * 64:(di + 1) * 64, :]
                    if layer == 0:
                        dst = h0_buf[di * 64:(di + 1) * 64,
                                     t_proc * B + hf * HB:t_proc * B + hf * HB + HB]
                        nc.vector.scalar_tensor_tensor(dst, to_d, 1.0, tc_d,
                                                       ALU.add, ALU.mult)
                    else:
                        dst = h1[hf][0:64, di * HB:(di + 1) * HB]
                        nc.vector.scalar_tensor_tensor(dst, to_d, 1.0, tc_d,
                                                       ALU.add, ALU.mult)
                        if (di == 0 and s == s_steps - 1) or (di == 1 and s == 0):
                            lst = h1_last[di * 64:(di + 1) * 64,
                                          hf * HB:hf * HB + HB]
                            nc.vector.scalar_tensor_tensor(lst, to_d, 1.0, tc_d,
                                                           ALU.add, ALU.mult)

            for _rep in range(reps):
                load_chunk("f", 0)
                load_chunk("r", 0)
                if n_ch > 1:
                    load_chunk("f", 1)
                    load_chunk("r", 1)
                for layer in (0, 1):
                    for hf in range(NH):
                        nc.vector.memset(cst[hf][:], 0.0)
                        if layer == 1:
                            nc.vector.memset(h1[hf][0:64, :], 0.0)
                            nc.vector.memset(h1[hf][64:65, :], 1.0)
                    for s in range(s_steps):
                        if layer == 0 and s % ct == 0:
                            k = s // ct + 2
                            if k < n_ch:
                                load_chunk("f", k)
                                load_chunk("r", k)
                        Ts = [None] * NH
                        for hf in range(NH):
                            G = phase_mm(layer, s, hf)
                            Ts[hf] = phase_gates(G, hf)
                            if hf >= 1:
                                tcp = phase_tanh_c(hf - 1)
                            phase_cup(Ts[hf], hf)
                            if hf >= 1:
                                phase_h(Ts[hf - 1], tcp, layer, s, hf - 1)
                        tcl = phase_tanh_c(NH - 1)
                        phase_h(Ts[NH - 1], tcl, layer, s, NH - 1)

                # ---- fc head ----
                pfc = psf.tile([C_OUT, B], f32, tag="pfc", name="pfc")
                nc.tensor.matmul(pfc[:], fcT_sb[:], h1_last[:],
                                 start=True, stop=True)
                osb = wk.tile([C_OUT, B], f32, tag="osb", name="osb")
                nc.scalar.activation(osb[:], pfc[:], AF.Identity,
                                     bias=fcb_sb[:, 0:1])
                nc.sync.dma_start(out=out_d[:], in_=osb[:])

    _split_multi_waits(nc, mybir)
    return nc


_cached = {}


def kernel(**inputs):
    from concourse.bass_utils import run_bass_kernel_spmd

    if "nc" not in _cached:
        _cached["nc"] = build_nc(S)
    nc = _cached["nc"]

    shared = _host_prep(inputs)
    x = np.asarray(inputs["x"], np.float32)
    in_maps = []
    for c in range(NCORES):
        m = dict(shared)
        m["xT"] = _host_xT(x[c * B:(c + 1) * B])
        in_maps.append(m)

    res = run_bass_kernel_spmd(nc, in_maps, list(range(NCORES)))
    out = np.concatenate([r["out"].T for r in res.results], axis=0)
    return np.ascontiguousarray(out.astype(np.float32))


# revision 16
# speedup vs baseline: 795.9410x; 1.3728x over previous
"""Trainium2 Bass kernel for 2-layer bidirectional LSTM (B=1024,S=256,F=16,H=64).

Sharding: batch data-parallel across 8 cores (128 batch rows each), weights
replicated. Per core the 128-row batch is split into two 64-row halves that
run as independent, staggered recurrence chains to hide per-step dependency
latency behind engine work.

All four gates of both directions are evaluated with a SINGLE Tanh ACT op per
half-step using sigma(a) = (1 + tanh(a/2)) / 2: the i/f/o gate rows of every
weight matrix and bias are pre-scaled by 1/2 on the host, so the PSUM gate
tile [128, 256] = [X_f | X_r | Y_f | Y_r] (X = [f;i] rows, Y = [o;g] rows)
goes through one tanh. With c stored as 2c and h stored as 2h:

    cb = (t_i + 1) * t_g            (DVE scalar_tensor_tensor)
    ca = (t_f + 1) * c2             (DVE scalar_tensor_tensor)
    c2' = 0.5 * ca + cb             (DVE scalar_tensor_tensor)
    tc = tanh(0.5 * c2')            (ACT, scale operand)
    h2 = (t_o + 1) * tc             (DVE scalar_tensor_tensor, fp16 out)

The 0.5 compensation for the doubled h is folded into every consumer weight
matrix on the host. Matmul inputs (x, h, weights) are fp16: 1 PE cycle/row
vs fp32's 4. Biases ride in the matmuls via a constant-one row appended to
the layer-0 x slab (K=17) and to the layer-1 h state tile (K=65), so the ACT
needs no bias operand and gates of both directions share one instruction.
"""
import numpy as np

H = 64
B = 128          # batch per core
HB = 64          # batch half per chain
S = 256
F = 16
CT = 32          # timesteps per streamed x chunk
NCORES = 8
FULL_B = 1024
C_OUT = 3


def _prep_dir(w_ih, w_hh, b_ih, b_hh, layer):
    """Per-direction lhsT stacks (fp16) with the all-tanh row scaling.

    PyTorch gate row order i,f,g,o. X-stack = [f;i], Y-stack = [o;g].
    Row scale 1/2 on i,f,o (sigma->tanh argument halving); g unscaled.
    Input-side scale 1/2 on everything that consumes a doubled h: all w_hh,
    and layer-1's w_ih (its input is the doubled h0).
    """
    w_ih = np.asarray(w_ih, np.float64)
    w_hh = np.asarray(w_hh, np.float64)
    b = np.asarray(b_ih, np.float64) + np.asarray(b_hh, np.float64)
    permX = np.r_[64:128, 0:64]        # [f; i]
    permY = np.r_[192:256, 128:192]    # [o; g]
    rsX = np.full((128, 1), 0.5)
    rsY = np.concatenate([np.full((64, 1), 0.5), np.ones((64, 1))])
    ih_scale = 1.0 if layer == 0 else 0.5
    out = {}
    for nm, perm, rs in (("X", permX, rsX), ("Y", permY, rsY)):
        wi = (ih_scale * rs * w_ih[perm]).T        # [din, 128]
        wh = (0.5 * rs * w_hh[perm]).T             # [64, 128]
        bb = (rs[:, 0] * b[perm])[None, :]         # [1, 128]
        if layer == 0:
            out[f"w{nm}"] = np.ascontiguousarray(
                np.concatenate([wi, bb], axis=0)).astype(np.float16)   # [17,128]
            out[f"u{nm}"] = np.ascontiguousarray(wh).astype(np.float16)  # [64,128]
        else:
            out[f"w{nm}"] = np.ascontiguousarray(wi).astype(np.float16)  # [128,128]
            out[f"u{nm}"] = np.ascontiguousarray(
                np.concatenate([wh, bb], axis=0)).astype(np.float16)   # [65,128]
    return out


def _host_prep(inputs):
    """Shared (weight) DRAM arrays, replicated to every core."""
    d = {}
    for layer in (0, 1):
        for di, suf in enumerate(("", "r")):
            p = _prep_dir(inputs[f"w_ih_l{layer}{suf}"],
                          inputs[f"w_hh_l{layer}{suf}"],
                          inputs[f"b_ih_l{layer}{suf}"],
                          inputs[f"b_hh_l{layer}{suf}"], layer)
            dch = "f" if di == 0 else "r"
            for nm in ("X", "Y"):
                d[f"w{layer}{nm}{dch}"] = p[f"w{nm}"]
                d[f"u{layer}{nm}{dch}"] = p[f"u{nm}"]
    # Layer-0 recurrent weights dir-stacked on partitions so the dir-r
    # matmul's lhsT shares the rhs base partition (h0_buf rows 64:128).
    for nm in ("X", "Y"):
        d[f"u0{nm}c"] = np.ascontiguousarray(
            np.concatenate([d.pop(f"u0{nm}f"), d.pop(f"u0{nm}r")], axis=0))
    d["fcT"] = np.ascontiguousarray(
        (0.5 * np.asarray(inputs["fc_w"], np.float64)).T).astype(np.float16)
    d["fcb"] = np.asarray(inputs["fc_b"], np.float32).reshape(C_OUT, 1)
    return d


def _host_xT(x_core):
    """x_core [B, S, F] -> [F+1, S*B] fp16, col = t*B + b, last row = 1."""
    xt = np.asarray(x_core, np.float32).transpose(2, 1, 0).reshape(F, -1)
    ones = np.ones((1, xt.shape[1]), np.float32)
    return np.concatenate([xt, ones], axis=0).astype(np.float16)


def _patch_tile_drain():
    """This container's walrus rejects instructions carrying multiple sync
    waits ("Too many sync wait commands") — chunk the kernel-tail drain's
    global-clock waits into one drain instruction per semaphore."""
    import concourse.tile as tile
    from concourse.vector_clock import ScopedClock, VectorClock
    if getattr(tile.TileContext, "_drain_patched", False):
        return
    def patched_drain(self, tick_clock, wait_clock):
        gc = tick_clock.global_clock
        n = len(gc)
        procs = [i for i in range(n) if gc[i] > 0]
        chunks = [[p] for p in procs] or [[]]
        for ch in chunks:
            vec = [0] * n
            for p in ch:
                vec[p] = gc[p]
            d = self.nc.sync.drain()
            wait_clock.add_sem_waits(d.ins, ScopedClock({None: VectorClock(vec)}))
        self.nc.all_engine_barrier()
        popped = self.nc._tile_sem_poison_stack.pop()
        assert popped is self._sem_poison
        self.nc.clear_and_free_semaphores(list(self.sems.allocated().values()))
        self.nc.all_engine_barrier()
    tile.TileContext._drain_and_barrier = patched_drain
    tile.TileContext._drain_patched = True


def _split_multi_waits(nc, mybir):
    """This walrus build rejects instructions with more than one sync wait.
    Hoist extra waits onto same-engine NoOp instructions inserted immediately
    before the owning instruction (identical semantics: the engine is
    sequential, so waiting on the prior instruction slot is equivalent)."""
    for f in nc.m.functions:
        for bb in f.blocks:
            out = []
            changed = False
            for inst in bb.instructions:
                si = inst.sync_info
                waits = list(si.on_wait) if si is not None else []
                if len(waits) > 1:
                    changed = True
                    for w in waits[:-1]:
                        nop = mybir.InstNoOp(
                            name=nc.get_next_instruction_name(), ins=[], outs=[])
                        nop.engine = inst.engine
                        nop.sync_info = mybir.SyncInfo(on_wait=[w], on_update=[])
                        out.append(nop)
                    inst.sync_info = mybir.SyncInfo(
                        on_wait=[waits[-1]], on_update=list(si.on_update))
                out.append(inst)
            if changed:
                bb.instructions = out


def build_nc(s_steps=S, reps=1):
    """reps > 1 unrolls the whole computation `reps` times inside one NEFF
    (same tiles, same output), letting the timing harness amortize the
    per-execution runtime launch overhead; the result is identical."""
    import concourse.bass as bass
    import concourse.tile as tile
    from concourse import mybir
    _patch_tile_drain()

    f32 = mybir.dt.float32
    f16 = mybir.dt.float16
    AF = mybir.ActivationFunctionType
    ALU = mybir.AluOpType

    ct = min(CT, s_steps)
    n_ch = s_steps // ct
    nc = bass.Bass("TRN2", target_bir_lowering=False, debug=False)

    xT_d = nc.dram_tensor("xT", [F + 1, s_steps * B], f16, kind="ExternalInput")
    wnames = ([f"w0{nm}{dc}" for nm in "XY" for dc in "fr"]
              + [f"u0{nm}c" for nm in "XY"]
              + [f"w1{nm}{dc}" for nm in "XY" for dc in "fr"]
              + [f"u1{nm}{dc}" for nm in "XY" for dc in "fr"])
    wshape = {"w0": [F + 1, 128], "u0": [128, 128], "w1": [128, 128], "u1": [65, 128]}
    wd = {n: nc.dram_tensor(n, wshape[n[:2]], f16, kind="ExternalInput")
          for n in wnames}
    fcT_d = nc.dram_tensor("fcT", [128, C_OUT], f16, kind="ExternalInput")
    fcb_d = nc.dram_tensor("fcb", [C_OUT, 1], f32, kind="ExternalInput")
    out_d = nc.dram_tensor("out", [C_OUT, B], f32, kind="ExternalOutput")

    with tile.TileContext(nc) as tc:
        with tc.tile_pool(name="pers", bufs=1) as pers, \
             tc.tile_pool(name="xch", bufs=3) as xch, \
             tc.tile_pool(name="wk", bufs=2) as wk, \
             tc.tile_pool(name="ps", bufs=2, space="PSUM") as ps:

            h0_buf = pers.tile([128, s_steps * B], f16, tag="h0buf", name="h0_buf")
            h1 = [pers.tile([65, B], f16, tag=f"h1_{hf}", name=f"h1{hf}")
                  for hf in (0, 1)]
            cst = [pers.tile([64, B], f32, tag=f"c_{hf}", name=f"c{hf}")
                   for hf in (0, 1)]
            h1_last = pers.tile([128, B], f16, tag="h1l", name="h1_last")

            wsb = {}
            for n in wnames:
                t = pers.tile(wshape[n[:2]], f16, tag=f"w_{n}", name=f"w_{n}")
                nc.sync.dma_start(out=t[:], in_=wd[n][:])
                wsb[n] = t
            fcT_sb = pers.tile([128, C_OUT], f16, tag="fcT", name="fcT_sb")
            nc.sync.dma_start(out=fcT_sb[:], in_=fcT_d[:])
            fcb_sb = pers.tile([C_OUT, 1], f32, tag="fcb", name="fcb_sb")
            nc.sync.dma_start(out=fcb_sb[:], in_=fcb_d[:])

            # --- x chunk streaming (layer 0 only), one stream per direction.
            chunks = {}

            def load_chunk(dc, k):
                t = xch.tile([F + 1, ct * B], f16, tag=f"x{dc}", name=f"x{dc}{k}")
                if dc == "f":
                    lo = k * ct * B
                else:
                    lo = (s_steps - (k + 1) * ct) * B
                nc.sync.dma_start(out=t[:], in_=xT_d[:, lo:lo + ct * B])
                chunks[dc, k] = t

            def x_rhs(dc, t_proc, hf):
                k = t_proc // ct if dc == "f" else (s_steps - 1 - t_proc) // ct
                ch = chunks[dc, k]
                base = k * ct if dc == "f" else s_steps - (k + 1) * ct
                off = (t_proc - base) * B + hf * HB
                return ch[:, off:off + HB]

            # Column layout inside the gate PSUM tile [128, 4*HB]:
            # [X_f | X_r | Y_f | Y_r], X rows = [f;i], Y rows = [o;g].
            def phase_mm(layer, s, hf):
                # One PSUM accumulation group open at a time: each region's
                # ih (start) matmul is immediately followed by its hh (stop)
                # matmul — interleaved open groups in one bank lose the
                # earlier contribution.
                G = ps.tile([128, 4 * HB], f32, tag=f"G{hf}", name=f"G{hf}_{layer}_{s}")
                first_step = s == 0
                for di, dc in enumerate(("f", "r")):
                    t_proc = s if dc == "f" else s_steps - 1 - s
                    t_prev = s - 1 if dc == "f" else s_steps - s
                    for ni, nm in enumerate(("X", "Y")):
                        dst = G[:, (2 * ni + di) * HB:(2 * ni + di + 1) * HB]
                        if layer == 0:
                            rhs = x_rhs(dc, t_proc, hf)
                        else:
                            rhs = h0_buf[:, t_proc * B + hf * HB:
                                         t_proc * B + hf * HB + HB]
                        only = layer == 0 and first_step
                        nc.tensor.matmul(dst, wsb[f"w{layer}{nm}{dc}"][:], rhs,
                                         start=True, stop=only)
                        if only:
                            continue
                        if layer == 0:
                            rhs2 = h0_buf[di * 64:(di + 1) * 64,
                                          t_prev * B + hf * HB:
                                          t_prev * B + hf * HB + HB]
                            lhsT = wsb[f"u0{nm}c"][di * 64:(di + 1) * 64, :]
                        else:
                            rhs2 = h1[hf][:, di * HB:(di + 1) * HB]
                            lhsT = wsb[f"u1{nm}{dc}"][:]
                        nc.tensor.matmul(dst, lhsT, rhs2,
                                         start=False, stop=True)
                return G

            def phase_gates(G, hf):
                T = wk.tile([128, 4 * HB], f32, tag=f"T{hf}", name=f"T{hf}")
                nc.scalar.activation(T[:], G[:], AF.Tanh)
                return T

            def phase_cup(T, hf):
                ti = T[64:128, 0:B]
                tg = T[64:128, B:2 * B]
                u = wk.tile([64, B], f32, tag=f"u{hf}", name=f"u{hf}")
                nc.vector.scalar_tensor_tensor(u[:], ti, 1.0, tg,
                                               ALU.add, ALU.mult)
                tf = T[0:64, 0:B]
                ca = wk.tile([64, B], f32, tag=f"ca{hf}", name=f"ca{hf}")
                nc.vector.scalar_tensor_tensor(ca[:], tf, 1.0, cst[hf][:],
                                               ALU.add, ALU.mult)
                nc.vector.scalar_tensor_tensor(cst[hf][:], ca[:], 0.5, u[:],
                                               ALU.mult, ALU.add)

            def phase_tanh_c(hf):
                tcv = wk.tile([64, B], f32, tag=f"tc{hf}", name=f"tc{hf}")
                nc.scalar.activation(tcv[:], cst[hf][:], AF.Tanh, scale=0.5)
                return tcv

            def phase_h(T, tcv, layer, s, hf):
                for di in (0, 1):
                    t_proc = s if di == 0 else s_steps - 1 - s
                    to_d = T[0:64, 2 * HB + di * HB:2 * HB + (di + 1) * HB]
                    tc_d = tcv[:, di * HB:(di + 1) * HB]
                    if layer == 0:
                        dst = h0_buf[di * 64:(di + 1) * 64,
                                     t_proc * B + hf * HB:t_proc * B + hf * HB + HB]
                        nc.vector.scalar_tensor_tensor(dst, to_d, 1.0, tc_d,
                                                       ALU.add, ALU.mult)
                    else:
                        dst = h1[hf][0:64, di * HB:(di + 1) * HB]
                        nc.vector.scalar_tensor_tensor(dst, to_d, 1.0, tc_d,
                                                       ALU.add, ALU.mult)
                        if (di == 0 and s == s_steps - 1) or (di == 1 and s == 0):
                            lst = h1_last[di * 64:(di + 1) * 64,
                                          hf * HB:hf * HB + HB]
                            nc.vector.scalar_tensor_tensor(lst, to_d, 1.0, tc_d,
                                                           ALU.add, ALU.mult)

            for _rep in range(reps):
                load_chunk("f", 0)
                load_chunk("r", 0)
                if n_ch > 1:
                    load_chunk("f", 1)
                    load_chunk("r", 1)
                for layer in (0, 1):
                    for hf in (0, 1):
                        nc.vector.memset(cst[hf][:], 0.0)
                        if layer == 1:
                            nc.vector.memset(h1[hf][0:64, :], 0.0)
                            nc.vector.memset(h1[hf][64:65, :], 1.0)
                    for s in range(s_steps):
                        if layer == 0 and s % ct == 0:
                            k = s // ct + 2
                            if k < n_ch:
                                load_chunk("f", k)
                                load_chunk("r", k)
                        # half 0: matmuls + gates + cell update
                        G0 = phase_mm(layer, s, 0)
                        T0 = phase_gates(G0, 0)
                        phase_cup(T0, 0)
                        # half 1: matmuls + gates (ACT fills while DVE runs h0)
                        G1 = phase_mm(layer, s, 1)
                        T1 = phase_gates(G1, 1)
                        tc0 = phase_tanh_c(0)
                        phase_cup(T1, 1)
                        phase_h(T0, tc0, layer, s, 0)
                        tc1 = phase_tanh_c(1)
                        phase_h(T1, tc1, layer, s, 1)

                # ---- fc head ----
                pfc = ps.tile([C_OUT, B], f32, tag="pfc", name="pfc")
                nc.tensor.matmul(pfc[:], fcT_sb[:], h1_last[:],
                                 start=True, stop=True)
                osb = wk.tile([C_OUT, B], f32, tag="osb", name="osb")
                nc.scalar.activation(osb[:], pfc[:], AF.Identity,
                                     bias=fcb_sb[:, 0:1])
                nc.sync.dma_start(out=out_d[:], in_=osb[:])

    _split_multi_waits(nc, mybir)
    return nc


_cached = {}


def kernel(**inputs):
    from concourse.bass_utils import run_bass_kernel_spmd

    if "nc" not in _cached:
        _cached["nc"] = build_nc(S)
    nc = _cached["nc"]

    shared = _host_prep(inputs)
    x = np.asarray(inputs["x"], np.float32)
    in_maps = []
    for c in range(NCORES):
        m = dict(shared)
        m["xT"] = _host_xT(x[c * B:(c + 1) * B])
        in_maps.append(m)

    res = run_bass_kernel_spmd(nc, in_maps, list(range(NCORES)))
    out = np.concatenate([r["out"].T for r in res.results], axis=0)
    return np.ascontiguousarray(out.astype(np.float32))
